# revision 25
# baseline (speedup 1.0000x reference)
"""Dense causal transformer attention block on 8 Trainium2 NeuronCores.

Problem: out = CausalAttention(RoPE(x@wq, x@wk), x@wv) @ wo
  x [2, 4096, 2048], 16 heads x 128 dim, fp32 I/O.

Sharding: tensor-parallel over heads. Core c owns heads {2c, 2c+1}:
  - computes qT/kT ([head_dim, seq] layout, w-stationary matmuls, RoPE
    on-chip) and V ([seq, head_dim] layout, x-stationary matmuls — no
    PE transpose needed) for its heads from the host-pre-transposed xT,
  - runs causal attention in transposed form (scoresT = k @ qT so the
    softmax weights come out as the moving operand of the A@V matmul),
  - denominators via an all-ones [128,128] stationary matmul (comes out
    pre-broadcast across partitions), fast approximate reciprocal,
  - computes its partial output projection o_local @ wo[rows of its heads].
Host sums the 8 partial outputs (the wo row-parallel all-reduce).

Software pipelining: the projection matmul chains for query tile i+1 are
emitted in small units interleaved into the attention pair loop of tile i,
so the PE has ready work while the scalar engine's exp() of each score
pair is still in flight. Causal masks are folded in as et multiplies
split across GpSimd and Vector; pair order puts one mask-free pair first
so the masked pairs' exp+mask latency hides behind it.

Compute dtype bf16 (PE 1 cycle/row), accumulation fp32 in PSUM.
"""
import sys

for _p in ("/opt/trn_rl_repo",):
    if _p not in sys.path:
        sys.path.insert(0, _p)

import numpy as np
import ml_dtypes
from contextlib import ExitStack

import concourse.bass as bass
import concourse.tile as tile
from concourse import bacc, mybir
from concourse import bass_utils

B, S, D = 2, 4096, 2048
H, DH = 16, 128
HALF = DH // 2
NC = 8
HPC = H // NC          # heads per core = 2
DOUT = HPC * DH        # 256 local proj width
ROPE_BASE = 10000.0
SCALE = 1.0 / float(np.sqrt(DH))
SQ = 512               # query tile (free dim of scoresT)
SKB = 128              # key block (partitions of scoresT)
KM = D // 128          # 16 contraction blocks
NSQ = S // SQ          # 8 query tiles per batch
NB = B * NSQ           # 16 bodies
BF = mybir.dt.bfloat16
F32 = mybir.dt.float32
INTERLEAVE = True

_CACHED = {}


def _build():
    nc = bacc.Bacc("TRN2", target_bir_lowering=False, debug=False, num_devices=NC)

    xT = nc.dram_tensor("xT", [D, B * S], BF, kind="ExternalInput").ap()
    wq = nc.dram_tensor("wq", [D, DOUT], BF, kind="ExternalInput").ap()
    wk = nc.dram_tensor("wk", [D, DOUT], BF, kind="ExternalInput").ap()
    wv = nc.dram_tensor("wv", [D, DOUT], BF, kind="ExternalInput").ap()
    wo = nc.dram_tensor("wo", [DOUT, D], BF, kind="ExternalInput").ap()
    cosf = nc.dram_tensor("cosf", [DH, S], BF, kind="ExternalInput").ap()
    sins = nc.dram_tensor("sins", [DH, S], BF, kind="ExternalInput").ap()
    masks = nc.dram_tensor("masks", [SKB, 4 * SQ], BF, kind="ExternalInput").ap()
    ones = nc.dram_tensor("ones", [128, 128], BF, kind="ExternalInput").ap()
    outp = nc.dram_tensor("outp", [B * S, D], BF, kind="ExternalOutput").ap()

    with tile.TileContext(nc) as tc, ExitStack() as ctx:
        const = ctx.enter_context(tc.tile_pool(name="const", bufs=1))
        xpool = ctx.enter_context(tc.tile_pool(name="xpool", bufs=4))
        qkv = ctx.enter_context(tc.tile_pool(name="qkv", bufs=1))
        rope = ctx.enter_context(tc.tile_pool(name="rope", bufs=2))
        attn = ctx.enter_context(tc.tile_pool(name="attn", bufs=4))
        opool = ctx.enter_context(tc.tile_pool(name="opool", bufs=2))

        wq_sb = const.tile([128, KM * DOUT], BF, name="wq_sb")
        wk_sb = const.tile([128, KM * DOUT], BF, name="wk_sb")
        wv_sb = const.tile([128, KM * DOUT], BF, name="wv_sb")
        ones_sb = const.tile([128, 128], BF, name="ones_sb")
        nc.sync.dma_start(ones_sb[:], ones[:])
        cos_sb = const.tile([DH, S], BF, name="cos_sb")
        sin_sb = const.tile([DH, S], BF, name="sin_sb")  # rows 0-63 = -sin
        mask_sb = const.tile([SKB, 4 * SQ], BF, name="mask_sb")
        wo_sb = const.tile([128, HPC * D], BF, name="wo_sb")    # [p, h*2048+n]

        qT = [qkv.tile([128, S], BF, tag=f"qT{j}", name=f"qT{j}") for j in range(HPC)]
        kT = [qkv.tile([128, S], BF, tag=f"kT{j}", name=f"kT{j}") for j in range(HPC)]
        vsb = [qkv.tile([128, S], BF, tag=f"v{j}", name=f"v{j}") for j in range(HPC)]
        oT = [qkv.tile([128, S], BF, tag=f"oT{j}", name=f"oT{j}") for j in range(HPC)]

        def flat(i):
            return (i // NSQ, i % NSQ)

        xbts = {}

        def emit_x_dma(i, eng=None, halves=False):
            if i >= NB or i in xbts:
                return None
            b_, t_ = flat(i)
            s0_ = t_ * SQ
            tiles = [xpool.tile([128, 8 * SQ], BF, tag=f"xb{hh}", bufs=2,
                                name=f"xbt{hh}") for hh in range(2)]
            xbts[i] = tiles
            e = eng or nc.sync
            if not halves:
                for hh in range(2):
                    e.dma_start(
                        tiles[hh][:].rearrange("p (a n) -> p a n", n=SQ),
                        xT[hh * 1024:(hh + 1) * 1024,
                           b_ * S + s0_: b_ * S + s0_ + SQ]
                        .rearrange("(a p) n -> p a n", p=128))
                return None

            # half-tile DMA closures the caller interleaves with the
            # weight DMAs (first proj matmuls start after ~0.5 MB)
            def part(hh, ha):
                e.dma_start(
                    tiles[hh][:, ha * 4 * SQ:(ha + 1) * 4 * SQ]
                    .rearrange("p (a n) -> p a n", n=SQ),
                    xT[hh * 1024 + ha * 512: hh * 1024 + (ha + 1) * 512,
                       b_ * S + s0_: b_ * S + s0_ + SQ]
                    .rearrange("(a p) n -> p a n", p=128))
            return part

        with tc.tile_pool(name="psm", bufs=1, space="PSUM") as psm:
            # PSUM budget (8 banks): pqk 1, pv 1, pscr 2x2, po 1, pd 1.
            # Out-proj pf tiles borrow the pscr slots.

            def proj_units(i):
                """Generator: proj PE work for body i in ~1us units.

                Unit order q(j)->v-sub->k(j)->v-sub keeps an independent
                PSUM bank between the chains that reuse the pqk bank, so
                the RoPE (DVE) read of the previous chain's accumulator
                never stalls the PE.
                """
                b_, t_ = flat(i)
                s0_ = t_ * SQ
                xbt = xbts[i]

                def qk_chain(j, w_sb, dstt):
                    pp = psm.tile([128, SQ], F32, tag="pqk", name="pp")
                    for u0 in range(0, KM, 4):
                        for km in range(u0, u0 + 4):
                            nc.tensor.matmul(
                                pp[:],
                                w_sb[:, km * DOUT + j * DH:
                                     km * DOUT + (j + 1) * DH],
                                xbt[km // 8][:, (km % 8) * SQ:
                                             (km % 8 + 1) * SQ],
                                start=km == 0, stop=km == KM - 1)
                        yield
                    rt = rope.tile([128, SQ], F32, tag="rot", name="rt")
                    nc.vector.tensor_mul(
                        rt[0:HALF, :], pp[HALF:128, :],
                        sin_sb[0:HALF, s0_:s0_ + SQ])
                    nc.vector.tensor_mul(
                        rt[HALF:128, :], pp[0:HALF, :],
                        sin_sb[HALF:128, s0_:s0_ + SQ])
                    m1 = rope.tile([128, SQ], F32, tag="m1", name="m1")
                    nc.vector.tensor_mul(m1[:], pp[:], cos_sb[:, s0_:s0_ + SQ])
                    nc.vector.tensor_add(dstt[:, s0_:s0_ + SQ], m1[:], rt[:])
                    yield

                def v_sub(sb):
                    # x-stationary: out = x_blk.T @ wv -> [seq 128, dh 256]
                    pvv = psm.tile([128, DOUT], F32, tag="pv", name="pvv")
                    for km in range(KM):
                        nc.tensor.matmul(
                            pvv[:],
                            xbt[km // 8][:, (km % 8) * SQ + sb * 128:
                                         (km % 8) * SQ + (sb + 1) * 128],
                            wv_sb[:, km * DOUT:(km + 1) * DOUT],
                            start=km == 0, stop=km == KM - 1)
                        if km % 8 == 7:
                            yield
                    blk = 4 * t_ + sb
                    for j in range(HPC):
                        dst = vsb[j][:, blk * 128:(blk + 1) * 128]
                        if j == 0:
                            nc.vector.tensor_copy(dst, pvv[:, 0:128])
                        else:
                            nc.scalar.copy(dst, pvv[:, 128:256])
                    yield

                yield from qk_chain(0, wq_sb, qT[0])
                yield from v_sub(0)
                yield from qk_chain(0, wk_sb, kT[0])
                yield from v_sub(1)
                yield from qk_chain(1, wq_sb, qT[1])
                yield from v_sub(2)
                yield from qk_chain(1, wk_sb, kT[1])
                yield from v_sub(3)

            def pull(gen, n):
                if gen is None:
                    return None
                for _ in range(n):
                    try:
                        next(gen)
                    except StopIteration:
                        return None
                return gen

            for i_flat in range(NB):
                b, t = flat(i_flat)
                s0 = t * SQ
                if i_flat == 0:
                    # startup: interleave x(t0) halves with wq halves so the
                    # first proj chain starts after ~0.5 MB of DMA; the rest
                    # follows in need order.
                    xpart = emit_x_dma(0, halves=True)

                    def wq_half(wh):
                        nc.sync.dma_start(
                            wq_sb[:, wh * 8 * DOUT:(wh + 1) * 8 * DOUT]
                            .rearrange("p (a n) -> p a n", n=DOUT),
                            wq[wh * 1024:(wh + 1) * 1024, :]
                            .rearrange("(a p) n -> p a n", p=128))
                    xpart(0, 0)
                    wq_half(0)
                    xpart(0, 1)
                    wq_half(1)
                    xpart(1, 0)
                    xpart(1, 1)
                    nc.sync.dma_start(cos_sb[:], cosf[:])
                    nc.sync.dma_start(sin_sb[:], sins[:])
                    nc.sync.dma_start(
                        wv_sb[:].rearrange("p (a n) -> p a n", n=DOUT),
                        wv.rearrange("(a p) n -> p a n", p=128))
                    nc.sync.dma_start(
                        wk_sb[:].rearrange("p (a n) -> p a n", n=DOUT),
                        wk.rearrange("(a p) n -> p a n", p=128))
                    nc.sync.dma_start(mask_sb[:], masks[:])
                    emit_x_dma(1)
                    nc.sync.dma_start(
                        wo_sb[:].rearrange("p (a n) -> p a n", n=D),
                        wo.rearrange("(a p) n -> p a n", p=128))
                    # body 0's proj runs eagerly
                    g = proj_units(0)
                    while pull(g, 1) is not None:
                        pass
                    xbts.pop(0)
                else:
                    emit_x_dma(i_flat + 1, eng=nc.scalar)
                    if t == 0:
                        # batch boundary: body NSQ's proj was NOT pipelined
                        # into body NSQ-1 (its qT/kT/vsb writes for s0=0
                        # would clobber regions the previous batch's last
                        # attention still reads) — run it eagerly here.
                        g = proj_units(i_flat)
                        while pull(g, 1) is not None:
                            pass
                        xbts.pop(i_flat)

                # proj of the NEXT body: emitted in units interleaved into
                # this body's attention pair loop (fills exp-latency bubbles)
                # — never across the batch boundary.
                nxt = None
                if i_flat + 1 < NB and (i_flat + 1) % NSQ != 0:
                    nxt = proj_units(i_flat + 1)
                if not INTERLEAVE and nxt is not None:
                    while pull(nxt, 1) is not None:
                        pass
                    xbts.pop(i_flat + 1)
                    nxt = None

                # --- causal attention for this query tile ---------------
                for j in range(HPC):
                    nblk = 4 * t + 4
                    npair = nblk // 2
                    nquad = npair // 2
                    po = psm.tile([128, SQ], F32, tag="po", name="po")
                    pd = psm.tile([128, SQ], F32, tag="pd", name="pd")
                    # one mask-free pair first (needs only the exp), then
                    # the diagonal (masked) pairs, then the rest
                    if t == 0:
                        order = [0, 1]
                    else:
                        order = [0, 2 * t, 2 * t + 1] + list(range(1, 2 * t))
                    prev_et = None
                    for idx, p in enumerate(order):
                        pscr = psm.tile([128, 2 * SQ], F32, tag="pscr",
                                        bufs=2, name="pscr")
                        for h in range(2):
                            u = 2 * p + h
                            nc.tensor.matmul(
                                pscr[:, h * SQ:(h + 1) * SQ],
                                kT[j][:, u * SKB:(u + 1) * SKB],
                                qT[j][:, s0:s0 + SQ], start=True, stop=True,
                                skip_group_check=True)
                        et = attn.tile([128, 2 * SQ], BF, tag="et", bufs=4,
                                       name="et")
                        nc.scalar.activation(
                            et[:], pscr[:], mybir.ActivationFunctionType.Exp,
                            scale=SCALE)
                        if 2 * p >= 4 * t:  # pair on the diagonal band
                            r = 2 * p - 4 * t   # 0 or 2
                            nc.gpsimd.tensor_mul(
                                et[:, 0:SQ], et[:, 0:SQ],
                                mask_sb[:, r * SQ:(r + 1) * SQ])
                            nc.vector.tensor_mul(
                                et[:, SQ:2 * SQ], et[:, SQ:2 * SQ],
                                mask_sb[:, (r + 1) * SQ:(r + 2) * SQ])
                        # PE filler while exp/mask are in flight
                        nxt = pull(nxt, 2 if idx == 0 else 1)
                        for h in range(2):
                            nc.tensor.matmul(
                                po[:], vsb[j][:, (2 * p + h) * 128:
                                              (2 * p + h + 1) * 128],
                                et[:, h * SQ:(h + 1) * SQ],
                                start=idx == 0 and h == 0,
                                stop=idx == npair - 1 and h == 1)
                        if idx % 2 == 1:
                            # denominator partial sums on GpSimd: keeps the
                            # DVE queue short so RoPE (which gates the
                            # pipelined proj units) is never delayed
                            qi = idx // 2
                            qs = attn.tile([128, 2 * SQ], BF, tag="qs",
                                           bufs=2, name="qs")
                            nc.gpsimd.tensor_add(qs[:], prev_et[:], et[:])
                            qs2 = attn.tile([128, SQ], BF, tag="qs2",
                                            bufs=2, name="qs2")
                            nc.gpsimd.tensor_add(
                                qs2[:], qs[:, 0:SQ], qs[:, SQ:2 * SQ])
                            nc.tensor.matmul(
                                pd[:], ones_sb[:], qs2[:],
                                start=qi == 0, stop=qi == nquad - 1)
                        prev_et = et
                    rec = attn.tile([128, SQ], F32, tag="rec", bufs=2,
                                    name="rec")
                    nc.vector.reciprocal_approx_fast(rec[:], pd[:])
                    nc.vector.tensor_mul(oT[j][:, s0:s0 + SQ], po[:], rec[:])

                # drain leftover proj units before the out-proj burst
                if nxt is not None:
                    while pull(nxt, 1) is not None:
                        pass
                    xbts.pop(i_flat + 1)

                # --- out-proj for the 4 seq blocks completed at t --------
                # pf tiles borrow the pscr tag's slots; the 4 n-blocks land
                # in one [128, 2048] SBUF tile -> one DMA per m-block.
                last_body = i_flat == NB - 1
                for m in range(4 * t, 4 * t + 4):
                    ob = opool.tile([128, D], BF, tag="ob", bufs=2,
                                    name="ob")
                    for n in range(D // 512):
                        pf = psm.tile([128, 512], F32, tag="pscr", bufs=2,
                                      name="pf")
                        for jj in range(HPC):
                            nc.tensor.matmul(
                                pf[:], oT[jj][:, m * 128:(m + 1) * 128],
                                wo_sb[:, jj * D + n * 512:
                                      jj * D + (n + 1) * 512],
                                start=jj == 0, stop=jj == HPC - 1)
                        if (m + n) % 2 == 0:
                            nc.vector.tensor_copy(
                                ob[:, n * 512:(n + 1) * 512], pf[:])
                        else:
                            nc.scalar.copy(
                                ob[:, n * 512:(n + 1) * 512], pf[:])
                        if last_body:
                            # tail: per-block DMAs overlap the remaining
                            # copies instead of waiting for all four
                            nc.sync.dma_start(
                                outp[b * S + m * 128: b * S + (m + 1) * 128,
                                     n * 512:(n + 1) * 512],
                                ob[:, n * 512:(n + 1) * 512])
                    if not last_body:
                        nc.sync.dma_start(
                            outp[b * S + m * 128: b * S + (m + 1) * 128, :],
                            ob[:])

    nc.compile()
    return nc


def _host_inputs(x, wq, wk, wv, wo, cos, sin):
    bf16 = ml_dtypes.bfloat16
    xT = np.ascontiguousarray(x.reshape(B * S, D).T).astype(bf16)

    cos = np.asarray(cos, dtype=np.float32)        # [S, 64]
    sin = np.asarray(sin, dtype=np.float32)
    cosf = np.ascontiguousarray(
        np.concatenate([cos, cos], axis=1).T).astype(bf16)   # [128, S]
    sins = np.concatenate([-sin, sin], axis=1).T   # rows 0-63 negated
    sins = np.ascontiguousarray(sins).astype(bf16)

    i = np.arange(SKB)[:, None]
    jj = np.arange(SQ)[None, :]
    masks = np.concatenate(
        [(i + r * SKB <= jj) for r in range(4)], axis=1).astype(bf16)
    ones = np.ones((128, 128), dtype=bf16)

    in_maps = []
    for c in range(NC):
        lo = c * DOUT
        in_maps.append({
            "xT": xT,
            "wq": np.ascontiguousarray(wq[:, lo:lo + DOUT]).astype(bf16),
            "wk": np.ascontiguousarray(wk[:, lo:lo + DOUT]).astype(bf16),
            "wv": np.ascontiguousarray(wv[:, lo:lo + DOUT]).astype(bf16),
            "wo": np.ascontiguousarray(wo[lo:lo + DOUT, :]).astype(bf16),
            "cosf": cosf,
            "sins": sins,
            "masks": masks,
            "ones": ones,
        })
    return in_maps


def kernel(x, wq, wk, wv, wo, cos, sin, _trace=False, _tmpdir=None):
    if "nc" not in _CACHED:
        _CACHED["nc"] = _build()
    nc = _CACHED["nc"]
    in_maps = _host_inputs(
        np.asarray(x, dtype=np.float32), np.asarray(wq, dtype=np.float32),
        np.asarray(wk, dtype=np.float32), np.asarray(wv, dtype=np.float32),
        np.asarray(wo, dtype=np.float32), cos, sin)
    res = bass_utils.run_bass_kernel_spmd(
        nc, in_maps, core_ids=list(range(NC)), trace=_trace, tmpdir=_tmpdir)
    acc = np.zeros((B * S, D), dtype=np.float32)
    for c in range(NC):
        acc += res.results[c]["outp"].astype(np.float32)
    out = acc.reshape(B, S, D)
    if _trace:
        _CACHED["last_results"] = res
    return out


# revision 27
# speedup vs baseline: 1.2424x; 1.2424x over previous
"""Dense causal transformer attention block on 8 Trainium2 NeuronCores.

Problem: out = CausalAttention(RoPE(x@wq, x@wk), x@wv) @ wo
  x [2, 4096, 2048], 16 heads x 128 dim, fp32 I/O.

Sharding: tensor-parallel over heads. Core c owns heads {2c, 2c+1}:
  - computes qT/kT ([head_dim, seq] layout, w-stationary matmuls, RoPE
    on-chip) and V ([seq, head_dim] layout, x-stationary matmuls — no
    PE transpose needed) for its heads from the host-pre-transposed xT,
  - runs causal attention in transposed form (scoresT = k @ qT so the
    softmax weights come out as the moving operand of the A@V matmul),
  - denominators via an all-ones [128,128] stationary matmul (comes out
    pre-broadcast across partitions), fast approximate reciprocal,
  - computes its partial output projection o_local @ wo[rows of its heads].
Host sums the 8 partial outputs (the wo row-parallel all-reduce).

Software pipelining: the projection matmul chains for query tile i+1 are
emitted in small units interleaved into the attention pair loop of tile i,
so the PE has ready work while the scalar engine's exp() of each score
pair is still in flight. Causal masks are folded in as et multiplies
split across GpSimd and Vector; pair order puts one mask-free pair first
so the masked pairs' exp+mask latency hides behind it.

Compute dtype bf16 (PE 1 cycle/row), accumulation fp32 in PSUM.
"""
import sys

for _p in ("/opt/trn_rl_repo",):
    if _p not in sys.path:
        sys.path.insert(0, _p)

import numpy as np
import ml_dtypes
from contextlib import ExitStack

import concourse.bass as bass
import concourse.tile as tile
from concourse import bacc, mybir
from concourse import bass_utils

B, S, D = 2, 4096, 2048
H, DH = 16, 128
HALF = DH // 2
NC = 8
HPC = H // NC          # heads per core = 2
DOUT = HPC * DH        # 256 local proj width
ROPE_BASE = 10000.0
SCALE = 1.0 / float(np.sqrt(DH))
SQ = 512               # query tile (free dim of scoresT)
SKB = 128              # key block (partitions of scoresT)
KM = D // 128          # 16 contraction blocks
NSQ = S // SQ          # 8 query tiles per batch
NB = B * NSQ           # 16 bodies
BF = mybir.dt.bfloat16
F32 = mybir.dt.float32
INTERLEAVE = True

_CACHED = {}


def _build():
    nc = bacc.Bacc("TRN2", target_bir_lowering=False, debug=False, num_devices=NC)

    xT = nc.dram_tensor("xT", [D, B * S], BF, kind="ExternalInput").ap()
    wq = nc.dram_tensor("wq", [D, DOUT], BF, kind="ExternalInput").ap()
    wk = nc.dram_tensor("wk", [D, DOUT], BF, kind="ExternalInput").ap()
    wv = nc.dram_tensor("wv", [D, DOUT], BF, kind="ExternalInput").ap()
    wo = nc.dram_tensor("wo", [DOUT, D], BF, kind="ExternalInput").ap()
    cosf = nc.dram_tensor("cosf", [DH, S], BF, kind="ExternalInput").ap()
    sins = nc.dram_tensor("sins", [DH, S], BF, kind="ExternalInput").ap()
    masks = nc.dram_tensor("masks", [SKB, 4 * SQ], BF, kind="ExternalInput").ap()
    ones = nc.dram_tensor("ones", [128, 128], BF, kind="ExternalInput").ap()
    outp = nc.dram_tensor("outp", [B * S, D], BF, kind="ExternalOutput").ap()

    with tile.TileContext(nc) as tc, ExitStack() as ctx:
        const = ctx.enter_context(tc.tile_pool(name="const", bufs=1))
        xpool = ctx.enter_context(tc.tile_pool(name="xpool", bufs=4))
        qkv = ctx.enter_context(tc.tile_pool(name="qkv", bufs=1))
        rope = ctx.enter_context(tc.tile_pool(name="rope", bufs=2))
        attn = ctx.enter_context(tc.tile_pool(name="attn", bufs=4))
        opool = ctx.enter_context(tc.tile_pool(name="opool", bufs=2))

        wq_sb = const.tile([128, KM * DOUT], BF, name="wq_sb")
        wk_sb = const.tile([128, KM * DOUT], BF, name="wk_sb")
        wv_sb = const.tile([128, KM * DOUT], BF, name="wv_sb")
        ones_sb = const.tile([128, 128], BF, name="ones_sb")
        nc.sync.dma_start(ones_sb[:], ones[:])
        cos_sb = const.tile([DH, S], BF, name="cos_sb")
        sin_sb = const.tile([DH, S], BF, name="sin_sb")  # rows 0-63 = -sin
        mask_sb = const.tile([SKB, 4 * SQ], BF, name="mask_sb")
        wo_sb = const.tile([128, HPC * D], BF, name="wo_sb")    # [p, h*2048+n]

        qT = [qkv.tile([128, S], BF, tag=f"qT{j}", name=f"qT{j}") for j in range(HPC)]
        kT = [qkv.tile([128, S], BF, tag=f"kT{j}", name=f"kT{j}") for j in range(HPC)]
        vsb = [qkv.tile([128, S], BF, tag=f"v{j}", name=f"v{j}") for j in range(HPC)]
        oT = [qkv.tile([128, S], BF, tag=f"oT{j}", name=f"oT{j}") for j in range(HPC)]

        def flat(i):
            return (i // NSQ, i % NSQ)

        xbts = {}

        def emit_x_dma(i, eng=None, halves=False):
            if i >= NB or i in xbts:
                return None
            b_, t_ = flat(i)
            s0_ = t_ * SQ
            tiles = [xpool.tile([128, 8 * SQ], BF, tag=f"xb{hh}", bufs=2,
                                name=f"xbt{hh}") for hh in range(2)]
            xbts[i] = tiles
            e = eng or nc.sync
            if not halves:
                for hh in range(2):
                    e.dma_start(
                        tiles[hh][:].rearrange("p (a n) -> p a n", n=SQ),
                        xT[hh * 1024:(hh + 1) * 1024,
                           b_ * S + s0_: b_ * S + s0_ + SQ]
                        .rearrange("(a p) n -> p a n", p=128))
                return None

            # half-tile DMA closures the caller interleaves with the
            # weight DMAs (first proj matmuls start after ~0.5 MB)
            def part(hh, ha):
                e.dma_start(
                    tiles[hh][:, ha * 4 * SQ:(ha + 1) * 4 * SQ]
                    .rearrange("p (a n) -> p a n", n=SQ),
                    xT[hh * 1024 + ha * 512: hh * 1024 + (ha + 1) * 512,
                       b_ * S + s0_: b_ * S + s0_ + SQ]
                    .rearrange("(a p) n -> p a n", p=128))
            return part

        with tc.tile_pool(name="psm", bufs=1, space="PSUM") as psm:
            # PSUM budget (8 banks): pqk 1, pv 1, pscr 2x2, po 1, pd 1.
            # Out-proj pf tiles borrow the pscr slots.

            def proj_units(i):
                """Generator: proj PE work for body i in ~1us units.

                Unit order q(j)->v-sub->k(j)->v-sub keeps an independent
                PSUM bank between the chains that reuse the pqk bank, so
                the RoPE (DVE) read of the previous chain's accumulator
                never stalls the PE.
                """
                b_, t_ = flat(i)
                s0_ = t_ * SQ
                xbt = xbts[i]

                def qk_chain(j, w_sb, dstt):
                    pp = psm.tile([128, SQ], F32, tag="pqk", name="pp")
                    for u0 in range(0, KM, 4):
                        for km in range(u0, u0 + 4):
                            nc.tensor.matmul(
                                pp[:],
                                w_sb[:, km * DOUT + j * DH:
                                     km * DOUT + (j + 1) * DH],
                                xbt[km // 8][:, (km % 8) * SQ:
                                             (km % 8 + 1) * SQ],
                                start=km == 0, stop=km == KM - 1)
                        yield
                    rt = rope.tile([128, SQ], F32, tag="rot", name="rt")
                    nc.vector.tensor_mul(
                        rt[0:HALF, :], pp[HALF:128, :],
                        sin_sb[0:HALF, s0_:s0_ + SQ])
                    nc.vector.tensor_mul(
                        rt[HALF:128, :], pp[0:HALF, :],
                        sin_sb[HALF:128, s0_:s0_ + SQ])
                    m1 = rope.tile([128, SQ], F32, tag="m1", name="m1")
                    nc.vector.tensor_mul(m1[:], pp[:], cos_sb[:, s0_:s0_ + SQ])
                    nc.vector.tensor_add(dstt[:, s0_:s0_ + SQ], m1[:], rt[:])
                    yield

                def v_sub(sb):
                    # x-stationary: out = x_blk.T @ wv -> [seq 128, dh 256]
                    pvv = psm.tile([128, DOUT], F32, tag="pv", name="pvv")
                    for km in range(KM):
                        nc.tensor.matmul(
                            pvv[:],
                            xbt[km // 8][:, (km % 8) * SQ + sb * 128:
                                         (km % 8) * SQ + (sb + 1) * 128],
                            wv_sb[:, km * DOUT:(km + 1) * DOUT],
                            start=km == 0, stop=km == KM - 1)
                        if km % 8 == 7:
                            yield
                    blk = 4 * t_ + sb
                    for j in range(HPC):
                        dst = vsb[j][:, blk * 128:(blk + 1) * 128]
                        if j == 0:
                            nc.vector.tensor_copy(dst, pvv[:, 0:128])
                        else:
                            nc.scalar.copy(dst, pvv[:, 128:256])
                    yield

                yield from qk_chain(0, wq_sb, qT[0])
                yield from v_sub(0)
                yield from qk_chain(0, wk_sb, kT[0])
                yield from v_sub(1)
                yield from qk_chain(1, wq_sb, qT[1])
                yield from v_sub(2)
                yield from qk_chain(1, wk_sb, kT[1])
                yield from v_sub(3)

            def pull(gen, n):
                if gen is None:
                    return None
                for _ in range(n):
                    try:
                        next(gen)
                    except StopIteration:
                        return None
                return gen

            for i_flat in range(NB):
                b, t = flat(i_flat)
                s0 = t * SQ
                if i_flat == 0:
                    # startup: interleave x(t0) halves with wq halves so the
                    # first proj chain starts after ~0.5 MB of DMA; the rest
                    # follows in need order.
                    xpart = emit_x_dma(0, halves=True)

                    def wq_half(wh):
                        nc.sync.dma_start(
                            wq_sb[:, wh * 8 * DOUT:(wh + 1) * 8 * DOUT]
                            .rearrange("p (a n) -> p a n", n=DOUT),
                            wq[wh * 1024:(wh + 1) * 1024, :]
                            .rearrange("(a p) n -> p a n", p=128))
                    xpart(0, 0)
                    wq_half(0)
                    xpart(0, 1)
                    wq_half(1)
                    xpart(1, 0)
                    xpart(1, 1)
                    nc.sync.dma_start(cos_sb[:], cosf[:])
                    nc.sync.dma_start(sin_sb[:], sins[:])
                    nc.sync.dma_start(
                        wv_sb[:].rearrange("p (a n) -> p a n", n=DOUT),
                        wv.rearrange("(a p) n -> p a n", p=128))
                    nc.sync.dma_start(
                        wk_sb[:].rearrange("p (a n) -> p a n", n=DOUT),
                        wk.rearrange("(a p) n -> p a n", p=128))
                    nc.sync.dma_start(mask_sb[:], masks[:])
                    emit_x_dma(1)
                    nc.sync.dma_start(
                        wo_sb[:].rearrange("p (a n) -> p a n", n=D),
                        wo.rearrange("(a p) n -> p a n", p=128))
                    # body 0's proj runs eagerly
                    g = proj_units(0)
                    while pull(g, 1) is not None:
                        pass
                    xbts.pop(0)
                else:
                    emit_x_dma(i_flat + 1, eng=nc.scalar)
                    if t == 0:
                        # batch boundary: body NSQ's proj was NOT pipelined
                        # into body NSQ-1 (its qT/kT/vsb writes for s0=0
                        # would clobber regions the previous batch's last
                        # attention still reads) — run it eagerly here.
                        g = proj_units(i_flat)
                        while pull(g, 1) is not None:
                            pass
                        xbts.pop(i_flat)

                # proj of the NEXT body: emitted in units interleaved into
                # this body's attention pair loop (fills exp-latency bubbles)
                # — never across the batch boundary.
                nxt = None
                if i_flat + 1 < NB and (i_flat + 1) % NSQ != 0:
                    nxt = proj_units(i_flat + 1)
                if not INTERLEAVE and nxt is not None:
                    while pull(nxt, 1) is not None:
                        pass
                    xbts.pop(i_flat + 1)
                    nxt = None

                # --- causal attention for this query tile ---------------
                for j in range(HPC):
                    nblk = 4 * t + 4
                    npair = nblk // 2
                    nquad = npair // 2
                    po = psm.tile([128, SQ], F32, tag="po", name="po")
                    pd = psm.tile([128, SQ], F32, tag="pd", name="pd")
                    # one mask-free pair first (needs only the exp), then
                    # the diagonal (masked) pairs, then the rest
                    if t == 0:
                        order = [0, 1]
                    else:
                        order = [0, 2 * t, 2 * t + 1] + list(range(1, 2 * t))
                    prev_et = None
                    qs2_list = []
                    for idx, p in enumerate(order):
                        pscr = psm.tile([128, 2 * SQ], F32, tag="pscr",
                                        bufs=2, name="pscr")
                        for h in range(2):
                            u = 2 * p + h
                            nc.tensor.matmul(
                                pscr[:, h * SQ:(h + 1) * SQ],
                                kT[j][:, u * SKB:(u + 1) * SKB],
                                qT[j][:, s0:s0 + SQ], start=True, stop=True,
                                skip_group_check=True)
                        et = attn.tile([128, 2 * SQ], BF, tag="et", bufs=4,
                                       name="et")
                        nc.scalar.activation(
                            et[:], pscr[:], mybir.ActivationFunctionType.Exp,
                            scale=SCALE)
                        if 2 * p >= 4 * t:  # pair on the diagonal band
                            r = 2 * p - 4 * t   # 0 or 2
                            nc.gpsimd.tensor_mul(
                                et[:, 0:SQ], et[:, 0:SQ],
                                mask_sb[:, r * SQ:(r + 1) * SQ])
                            nc.vector.tensor_mul(
                                et[:, SQ:2 * SQ], et[:, SQ:2 * SQ],
                                mask_sb[:, (r + 1) * SQ:(r + 2) * SQ])
                        # PE filler while exp/mask are in flight
                        nxt = pull(nxt, 2 if idx == 0 else 1)
                        for h in range(2):
                            nc.tensor.matmul(
                                po[:], vsb[j][:, (2 * p + h) * 128:
                                              (2 * p + h + 1) * 128],
                                et[:, h * SQ:(h + 1) * SQ],
                                start=idx == 0 and h == 0,
                                stop=idx == npair - 1 and h == 1)
                        if idx % 2 == 1:
                            qs = attn.tile([128, 2 * SQ], BF, tag="qs",
                                           bufs=2, name="qs")
                            nc.vector.tensor_add(qs[:], prev_et[:], et[:])
                            qs2 = attn.tile([128, SQ], BF, tag="qs2",
                                            bufs=8, name="qs2")
                            nc.vector.tensor_add(
                                qs2[:], qs[:, 0:SQ], qs[:, SQ:2 * SQ])
                            qs2_list.append(qs2)
                        prev_et = et
                    # denominator matmuls batched at the end of the pair
                    # loop: inline they sit in the PE FIFO waiting on the
                    # DVE qs chain every quad; here only the last one can
                    # ever wait
                    for qi, qs2 in enumerate(qs2_list):
                        nc.tensor.matmul(
                            pd[:], ones_sb[:], qs2[:],
                            start=qi == 0, stop=qi == nquad - 1)
                    rec = attn.tile([128, SQ], F32, tag="rec", bufs=2,
                                    name="rec")
                    nc.vector.reciprocal_approx_fast(rec[:], pd[:])
                    nc.vector.tensor_mul(oT[j][:, s0:s0 + SQ], po[:], rec[:])

                # drain leftover proj units before the out-proj burst
                if nxt is not None:
                    while pull(nxt, 1) is not None:
                        pass
                    xbts.pop(i_flat + 1)

                # --- out-proj for the 4 seq blocks completed at t --------
                # pf tiles borrow the pscr tag's slots; the 4 n-blocks land
                # in one [128, 2048] SBUF tile -> one DMA per m-block.
                last_body = i_flat == NB - 1
                for m in range(4 * t, 4 * t + 4):
                    ob = opool.tile([128, D], BF, tag="ob", bufs=2,
                                    name="ob")
                    for n in range(D // 512):
                        pf = psm.tile([128, 512], F32, tag="pscr", bufs=2,
                                      name="pf")
                        for jj in range(HPC):
                            nc.tensor.matmul(
                                pf[:], oT[jj][:, m * 128:(m + 1) * 128],
                                wo_sb[:, jj * D + n * 512:
                                      jj * D + (n + 1) * 512],
                                start=jj == 0, stop=jj == HPC - 1)
                        if (m + n) % 2 == 0:
                            nc.vector.tensor_copy(
                                ob[:, n * 512:(n + 1) * 512], pf[:])
                        else:
                            nc.scalar.copy(
                                ob[:, n * 512:(n + 1) * 512], pf[:])
                        if last_body:
                            # tail: per-block DMAs overlap the remaining
                            # copies instead of waiting for all four
                            nc.sync.dma_start(
                                outp[b * S + m * 128: b * S + (m + 1) * 128,
                                     n * 512:(n + 1) * 512],
                                ob[:, n * 512:(n + 1) * 512])
                    if not last_body:
                        nc.sync.dma_start(
                            outp[b * S + m * 128: b * S + (m + 1) * 128, :],
                            ob[:])

    nc.compile()
    return nc


def _host_inputs(x, wq, wk, wv, wo, cos, sin):
    bf16 = ml_dtypes.bfloat16
    xT = np.ascontiguousarray(x.reshape(B * S, D).T).astype(bf16)

    cos = np.asarray(cos, dtype=np.float32)        # [S, 64]
    sin = np.asarray(sin, dtype=np.float32)
    cosf = np.ascontiguousarray(
        np.concatenate([cos, cos], axis=1).T).astype(bf16)   # [128, S]
    sins = np.concatenate([-sin, sin], axis=1).T   # rows 0-63 negated
    sins = np.ascontiguousarray(sins).astype(bf16)

    i = np.arange(SKB)[:, None]
    jj = np.arange(SQ)[None, :]
    masks = np.concatenate(
        [(i + r * SKB <= jj) for r in range(4)], axis=1).astype(bf16)
    ones = np.ones((128, 128), dtype=bf16)

    in_maps = []
    for c in range(NC):
        lo = c * DOUT
        in_maps.append({
            "xT": xT,
            "wq": np.ascontiguousarray(wq[:, lo:lo + DOUT]).astype(bf16),
            "wk": np.ascontiguousarray(wk[:, lo:lo + DOUT]).astype(bf16),
            "wv": np.ascontiguousarray(wv[:, lo:lo + DOUT]).astype(bf16),
            "wo": np.ascontiguousarray(wo[lo:lo + DOUT, :]).astype(bf16),
            "cosf": cosf,
            "sins": sins,
            "masks": masks,
            "ones": ones,
        })
    return in_maps


def kernel(x, wq, wk, wv, wo, cos, sin, _trace=False, _tmpdir=None):
    if "nc" not in _CACHED:
        _CACHED["nc"] = _build()
    nc = _CACHED["nc"]
    in_maps = _host_inputs(
        np.asarray(x, dtype=np.float32), np.asarray(wq, dtype=np.float32),
        np.asarray(wk, dtype=np.float32), np.asarray(wv, dtype=np.float32),
        np.asarray(wo, dtype=np.float32), cos, sin)
    res = bass_utils.run_bass_kernel_spmd(
        nc, in_maps, core_ids=list(range(NC)), trace=_trace, tmpdir=_tmpdir)
    acc = np.zeros((B * S, D), dtype=np.float32)
    for c in range(NC):
        acc += res.results[c]["outp"].astype(np.float32)
    out = acc.reshape(B, S, D)
    if _trace:
        _CACHED["last_results"] = res
    return out


# revision 30
# speedup vs baseline: 1.2475x; 1.0042x over previous
"""Dense causal transformer attention block on 8 Trainium2 NeuronCores.

Problem: out = CausalAttention(RoPE(x@wq, x@wk), x@wv) @ wo
  x [2, 4096, 2048], 16 heads x 128 dim, fp32 I/O.

Sharding: tensor-parallel over heads. Core c owns heads {2c, 2c+1}:
  - computes qT/kT ([head_dim, seq] layout, w-stationary matmuls, RoPE
    on-chip) and V ([seq, head_dim] layout, x-stationary matmuls — no
    PE transpose needed) for its heads from the host-pre-transposed xT,
  - runs causal attention in transposed form (scoresT = k @ qT so the
    softmax weights come out as the moving operand of the A@V matmul),
  - denominators via an all-ones [128,128] stationary matmul (comes out
    pre-broadcast across partitions), fast approximate reciprocal,
  - computes its partial output projection o_local @ wo[rows of its heads].
Host sums the 8 partial outputs (the wo row-parallel all-reduce).

Software pipelining: the projection matmul chains for query tile i+1 are
emitted in small units interleaved into the attention pair loop of tile i,
so the PE has ready work while the scalar engine's exp() of each score
pair is still in flight. Causal masks are folded in as et multiplies
split across GpSimd and Vector; pair order puts one mask-free pair first
so the masked pairs' exp+mask latency hides behind it.

Compute dtype bf16 (PE 1 cycle/row), accumulation fp32 in PSUM.
"""
import sys

for _p in ("/opt/trn_rl_repo",):
    if _p not in sys.path:
        sys.path.insert(0, _p)

import numpy as np
import ml_dtypes
from contextlib import ExitStack

import concourse.bass as bass
import concourse.tile as tile
from concourse import bacc, mybir
from concourse import bass_utils

B, S, D = 2, 4096, 2048
H, DH = 16, 128
HALF = DH // 2
NC = 8
HPC = H // NC          # heads per core = 2
DOUT = HPC * DH        # 256 local proj width
ROPE_BASE = 10000.0
SCALE = 1.0 / float(np.sqrt(DH))
SQ = 512               # query tile (free dim of scoresT)
SKB = 128              # key block (partitions of scoresT)
KM = D // 128          # 16 contraction blocks
NSQ = S // SQ          # 8 query tiles per batch
NB = B * NSQ           # 16 bodies
BF = mybir.dt.bfloat16
F32 = mybir.dt.float32
INTERLEAVE = True

_CACHED = {}


def _build():
    nc = bacc.Bacc("TRN2", target_bir_lowering=False, debug=False, num_devices=NC)

    xT = nc.dram_tensor("xT", [D, B * S], BF, kind="ExternalInput").ap()
    wq = nc.dram_tensor("wq", [D, DOUT], BF, kind="ExternalInput").ap()
    wk = nc.dram_tensor("wk", [D, DOUT], BF, kind="ExternalInput").ap()
    wv = nc.dram_tensor("wv", [D, DOUT], BF, kind="ExternalInput").ap()
    wo = nc.dram_tensor("wo", [DOUT, D], BF, kind="ExternalInput").ap()
    cosf = nc.dram_tensor("cosf", [DH, S], BF, kind="ExternalInput").ap()
    sins = nc.dram_tensor("sins", [DH, S], BF, kind="ExternalInput").ap()
    masks = nc.dram_tensor("masks", [SKB, 4 * SQ], BF, kind="ExternalInput").ap()
    ones = nc.dram_tensor("ones", [128, 128], BF, kind="ExternalInput").ap()
    outp = nc.dram_tensor("outp", [B * S, D], BF, kind="ExternalOutput").ap()

    with tile.TileContext(nc) as tc, ExitStack() as ctx:
        const = ctx.enter_context(tc.tile_pool(name="const", bufs=1))
        xpool = ctx.enter_context(tc.tile_pool(name="xpool", bufs=4))
        qkv = ctx.enter_context(tc.tile_pool(name="qkv", bufs=1))
        rope = ctx.enter_context(tc.tile_pool(name="rope", bufs=2))
        attn = ctx.enter_context(tc.tile_pool(name="attn", bufs=4))
        opool = ctx.enter_context(tc.tile_pool(name="opool", bufs=2))

        wq_sb = const.tile([128, KM * DOUT], BF, name="wq_sb")
        wk_sb = const.tile([128, KM * DOUT], BF, name="wk_sb")
        wv_sb = const.tile([128, KM * DOUT], BF, name="wv_sb")
        ones_sb = const.tile([128, 128], BF, name="ones_sb")
        nc.sync.dma_start(ones_sb[:], ones[:])
        cos_sb = const.tile([DH, S], BF, name="cos_sb")
        sin_sb = const.tile([DH, S], BF, name="sin_sb")  # rows 0-63 = -sin
        mask_sb = const.tile([SKB, 4 * SQ], BF, name="mask_sb")
        wo_sb = const.tile([128, HPC * D], BF, name="wo_sb")    # [p, h*2048+n]

        qT = [qkv.tile([128, S], BF, tag=f"qT{j}", name=f"qT{j}") for j in range(HPC)]
        kT = [qkv.tile([128, S], BF, tag=f"kT{j}", name=f"kT{j}") for j in range(HPC)]
        vsb = [qkv.tile([128, S], BF, tag=f"v{j}", name=f"v{j}") for j in range(HPC)]
        oT = [qkv.tile([128, S], BF, tag=f"oT{j}", name=f"oT{j}") for j in range(HPC)]

        def flat(i):
            return (i // NSQ, i % NSQ)

        xbts = {}

        def emit_x_dma(i, eng=None, halves=False):
            if i >= NB or i in xbts:
                return None
            b_, t_ = flat(i)
            s0_ = t_ * SQ
            tiles = [xpool.tile([128, 8 * SQ], BF, tag=f"xb{hh}", bufs=2,
                                name=f"xbt{hh}") for hh in range(2)]
            xbts[i] = tiles
            e = eng or nc.sync
            if not halves:
                for hh in range(2):
                    e.dma_start(
                        tiles[hh][:].rearrange("p (a n) -> p a n", n=SQ),
                        xT[hh * 1024:(hh + 1) * 1024,
                           b_ * S + s0_: b_ * S + s0_ + SQ]
                        .rearrange("(a p) n -> p a n", p=128))
                return None

            # half-tile DMA closures the caller interleaves with the
            # weight DMAs (first proj matmuls start after ~0.5 MB)
            def part(hh, ha):
                e.dma_start(
                    tiles[hh][:, ha * 4 * SQ:(ha + 1) * 4 * SQ]
                    .rearrange("p (a n) -> p a n", n=SQ),
                    xT[hh * 1024 + ha * 512: hh * 1024 + (ha + 1) * 512,
                       b_ * S + s0_: b_ * S + s0_ + SQ]
                    .rearrange("(a p) n -> p a n", p=128))
            return part

        with tc.tile_pool(name="psm", bufs=1, space="PSUM") as psm:
            # PSUM budget (8 banks): pqk 1, pv 1, pscr 2x2, po 1, pd 1.
            # Out-proj pf tiles borrow the pscr slots.

            def proj_units(i):
                """Generator: proj PE work for body i in ~1us units.

                Unit order q(j)->v-sub->k(j)->v-sub keeps an independent
                PSUM bank between the chains that reuse the pqk bank, so
                the RoPE (DVE) read of the previous chain's accumulator
                never stalls the PE.
                """
                b_, t_ = flat(i)
                s0_ = t_ * SQ
                xbt = xbts[i]

                def qk_chain(j, w_sb, dstt):
                    pp = psm.tile([128, SQ], F32, tag="pqk", name="pp")
                    for u0 in range(0, KM, 4):
                        for km in range(u0, u0 + 4):
                            nc.tensor.matmul(
                                pp[:],
                                w_sb[:, km * DOUT + j * DH:
                                     km * DOUT + (j + 1) * DH],
                                xbt[km // 8][:, (km % 8) * SQ:
                                             (km % 8 + 1) * SQ],
                                start=km == 0, stop=km == KM - 1)
                        yield
                    rt = rope.tile([128, SQ], F32, tag="rot", name="rt")
                    nc.vector.tensor_mul(
                        rt[0:HALF, :], pp[HALF:128, :],
                        sin_sb[0:HALF, s0_:s0_ + SQ])
                    nc.vector.tensor_mul(
                        rt[HALF:128, :], pp[0:HALF, :],
                        sin_sb[HALF:128, s0_:s0_ + SQ])
                    m1 = rope.tile([128, SQ], F32, tag="m1", name="m1")
                    nc.vector.tensor_mul(m1[:], pp[:], cos_sb[:, s0_:s0_ + SQ])
                    nc.vector.tensor_add(dstt[:, s0_:s0_ + SQ], m1[:], rt[:])
                    yield

                def v_sub(sb):
                    # x-stationary: out = x_blk.T @ wv -> [seq 128, dh 256]
                    pvv = psm.tile([128, DOUT], F32, tag="pv", name="pvv")
                    for km in range(KM):
                        nc.tensor.matmul(
                            pvv[:],
                            xbt[km // 8][:, (km % 8) * SQ + sb * 128:
                                         (km % 8) * SQ + (sb + 1) * 128],
                            wv_sb[:, km * DOUT:(km + 1) * DOUT],
                            start=km == 0, stop=km == KM - 1)
                        if km % 8 == 7:
                            yield
                    blk = 4 * t_ + sb
                    for j in range(HPC):
                        dst = vsb[j][:, blk * 128:(blk + 1) * 128]
                        if j == 0:
                            nc.vector.tensor_copy(dst, pvv[:, 0:128])
                        else:
                            nc.scalar.copy(dst, pvv[:, 128:256])
                    yield

                yield from qk_chain(0, wq_sb, qT[0])
                yield from v_sub(0)
                yield from qk_chain(0, wk_sb, kT[0])
                yield from v_sub(1)
                yield from qk_chain(1, wq_sb, qT[1])
                yield from v_sub(2)
                yield from qk_chain(1, wk_sb, kT[1])
                yield from v_sub(3)

            def pull(gen, n):
                if gen is None:
                    return None
                for _ in range(n):
                    try:
                        next(gen)
                    except StopIteration:
                        return None
                return gen

            projected = set()
            for i_flat in range(NB):
                b, t = flat(i_flat)
                s0 = t * SQ
                if i_flat == 0:
                    # startup: interleave x(t0) halves with wq halves so the
                    # first proj chain starts after ~0.5 MB of DMA; the rest
                    # follows in need order.
                    xpart = emit_x_dma(0, halves=True)

                    def wq_half(wh):
                        nc.sync.dma_start(
                            wq_sb[:, wh * 8 * DOUT:(wh + 1) * 8 * DOUT]
                            .rearrange("p (a n) -> p a n", n=DOUT),
                            wq[wh * 1024:(wh + 1) * 1024, :]
                            .rearrange("(a p) n -> p a n", p=128))
                    xpart(0, 0)
                    wq_half(0)
                    xpart(0, 1)
                    wq_half(1)
                    xpart(1, 0)
                    xpart(1, 1)
                    nc.sync.dma_start(cos_sb[:], cosf[:])
                    nc.sync.dma_start(sin_sb[:], sins[:])
                    nc.sync.dma_start(
                        wv_sb[:].rearrange("p (a n) -> p a n", n=DOUT),
                        wv.rearrange("(a p) n -> p a n", p=128))
                    nc.sync.dma_start(
                        wk_sb[:].rearrange("p (a n) -> p a n", n=DOUT),
                        wk.rearrange("(a p) n -> p a n", p=128))
                    nc.sync.dma_start(mask_sb[:], masks[:])
                    emit_x_dma(1)
                    nc.sync.dma_start(
                        wo_sb[:].rearrange("p (a n) -> p a n", n=D),
                        wo.rearrange("(a p) n -> p a n", p=128))
                else:
                    emit_x_dma(i_flat + 1, eng=nc.scalar)

                if i_flat not in projected:
                    # proj not pipelined into the previous body (body 0,
                    # body 1, and each batch-boundary body whose s0=0
                    # writes would clobber regions the previous batch's
                    # last attention still reads) — run it eagerly.
                    g = proj_units(i_flat)
                    while pull(g, 1) is not None:
                        pass
                    xbts.pop(i_flat)
                    projected.add(i_flat)

                # proj of the NEXT body: emitted in units interleaved into
                # this body's attention pair loop (fills exp-latency
                # bubbles) — not across the batch boundary, and not out of
                # body 0 (the next body's x DMA is still behind the
                # startup constants).
                nxt = None
                nxt_i = i_flat + 1
                if (INTERLEAVE and i_flat != 0 and nxt_i < NB
                        and nxt_i % NSQ != 0):
                    nxt = proj_units(nxt_i)
                    projected.add(nxt_i)

                # --- causal attention for this query tile ---------------
                for j in range(HPC):
                    nblk = 4 * t + 4
                    npair = nblk // 2
                    nquad = npair // 2
                    po = psm.tile([128, SQ], F32, tag="po", name="po")
                    pd = psm.tile([128, SQ], F32, tag="pd", name="pd")
                    # one mask-free pair first (needs only the exp), then
                    # the diagonal (masked) pairs, then the rest
                    if t == 0:
                        order = [0, 1]
                    else:
                        order = [0, 2 * t, 2 * t + 1] + list(range(1, 2 * t))
                    prev_et = None
                    qs2_list = []
                    for idx, p in enumerate(order):
                        pscr = psm.tile([128, 2 * SQ], F32, tag="pscr",
                                        bufs=2, name="pscr")
                        for h in range(2):
                            u = 2 * p + h
                            nc.tensor.matmul(
                                pscr[:, h * SQ:(h + 1) * SQ],
                                kT[j][:, u * SKB:(u + 1) * SKB],
                                qT[j][:, s0:s0 + SQ], start=True, stop=True,
                                skip_group_check=True)
                        et = attn.tile([128, 2 * SQ], BF, tag="et", bufs=4,
                                       name="et")
                        nc.scalar.activation(
                            et[:], pscr[:], mybir.ActivationFunctionType.Exp,
                            scale=SCALE)
                        if 2 * p >= 4 * t:  # pair on the diagonal band
                            r = 2 * p - 4 * t   # 0 or 2
                            nc.gpsimd.tensor_mul(
                                et[:, 0:SQ], et[:, 0:SQ],
                                mask_sb[:, r * SQ:(r + 1) * SQ])
                            nc.vector.tensor_mul(
                                et[:, SQ:2 * SQ], et[:, SQ:2 * SQ],
                                mask_sb[:, (r + 1) * SQ:(r + 2) * SQ])
                        # PE filler while exp/mask are in flight
                        nxt = pull(nxt, 2 if idx == 0 else 1)
                        for h in range(2):
                            nc.tensor.matmul(
                                po[:], vsb[j][:, (2 * p + h) * 128:
                                              (2 * p + h + 1) * 128],
                                et[:, h * SQ:(h + 1) * SQ],
                                start=idx == 0 and h == 0,
                                stop=idx == npair - 1 and h == 1)
                        if idx % 2 == 1:
                            qs = attn.tile([128, 2 * SQ], BF, tag="qs",
                                           bufs=2, name="qs")
                            nc.vector.tensor_add(qs[:], prev_et[:], et[:])
                            qs2 = attn.tile([128, SQ], BF, tag="qs2",
                                            bufs=8, name="qs2")
                            nc.vector.tensor_add(
                                qs2[:], qs[:, 0:SQ], qs[:, SQ:2 * SQ])
                            qs2_list.append(qs2)
                        prev_et = et
                    # denominator matmuls batched at the end of the pair
                    # loop: inline they sit in the PE FIFO waiting on the
                    # DVE qs chain every quad; here only the last one can
                    # ever wait
                    for qi, qs2 in enumerate(qs2_list):
                        nc.tensor.matmul(
                            pd[:], ones_sb[:], qs2[:],
                            start=qi == 0, stop=qi == nquad - 1)
                    rec = attn.tile([128, SQ], F32, tag="rec", bufs=2,
                                    name="rec")
                    nc.vector.reciprocal_approx_fast(rec[:], pd[:])
                    nc.vector.tensor_mul(oT[j][:, s0:s0 + SQ], po[:], rec[:])

                # drain leftover proj units before the out-proj burst
                if nxt is not None:
                    while pull(nxt, 1) is not None:
                        pass
                    xbts.pop(nxt_i)

                # --- out-proj for the 4 seq blocks completed at t --------
                # pf tiles borrow the pscr tag's slots; the 4 n-blocks land
                # in one [128, 2048] SBUF tile -> one DMA per m-block.
                last_body = i_flat == NB - 1
                for m in range(4 * t, 4 * t + 4):
                    ob = opool.tile([128, D], BF, tag="ob", bufs=2,
                                    name="ob")
                    for n in range(D // 512):
                        pf = psm.tile([128, 512], F32, tag="pscr", bufs=2,
                                      name="pf")
                        for jj in range(HPC):
                            nc.tensor.matmul(
                                pf[:], oT[jj][:, m * 128:(m + 1) * 128],
                                wo_sb[:, jj * D + n * 512:
                                      jj * D + (n + 1) * 512],
                                start=jj == 0, stop=jj == HPC - 1)
                        if (m + n) % 2 == 0:
                            nc.vector.tensor_copy(
                                ob[:, n * 512:(n + 1) * 512], pf[:])
                        else:
                            nc.scalar.copy(
                                ob[:, n * 512:(n + 1) * 512], pf[:])
                        if last_body:
                            # tail: per-block DMAs overlap the remaining
                            # copies instead of waiting for all four
                            nc.sync.dma_start(
                                outp[b * S + m * 128: b * S + (m + 1) * 128,
                                     n * 512:(n + 1) * 512],
                                ob[:, n * 512:(n + 1) * 512])
                    if not last_body:
                        nc.sync.dma_start(
                            outp[b * S + m * 128: b * S + (m + 1) * 128, :],
                            ob[:])

    nc.compile()
    return nc


def _host_inputs(x, wq, wk, wv, wo, cos, sin):
    bf16 = ml_dtypes.bfloat16
    xT = np.ascontiguousarray(x.reshape(B * S, D).T).astype(bf16)

    cos = np.asarray(cos, dtype=np.float32)        # [S, 64]
    sin = np.asarray(sin, dtype=np.float32)
    cosf = np.ascontiguousarray(
        np.concatenate([cos, cos], axis=1).T).astype(bf16)   # [128, S]
    sins = np.concatenate([-sin, sin], axis=1).T   # rows 0-63 negated
    sins = np.ascontiguousarray(sins).astype(bf16)

    i = np.arange(SKB)[:, None]
    jj = np.arange(SQ)[None, :]
    masks = np.concatenate(
        [(i + r * SKB <= jj) for r in range(4)], axis=1).astype(bf16)
    ones = np.ones((128, 128), dtype=bf16)

    in_maps = []
    for c in range(NC):
        lo = c * DOUT
        in_maps.append({
            "xT": xT,
            "wq": np.ascontiguousarray(wq[:, lo:lo + DOUT]).astype(bf16),
            "wk": np.ascontiguousarray(wk[:, lo:lo + DOUT]).astype(bf16),
            "wv": np.ascontiguousarray(wv[:, lo:lo + DOUT]).astype(bf16),
            "wo": np.ascontiguousarray(wo[lo:lo + DOUT, :]).astype(bf16),
            "cosf": cosf,
            "sins": sins,
            "masks": masks,
            "ones": ones,
        })
    return in_maps


def kernel(x, wq, wk, wv, wo, cos, sin, _trace=False, _tmpdir=None):
    if "nc" not in _CACHED:
        _CACHED["nc"] = _build()
    nc = _CACHED["nc"]
    in_maps = _host_inputs(
        np.asarray(x, dtype=np.float32), np.asarray(wq, dtype=np.float32),
        np.asarray(wk, dtype=np.float32), np.asarray(wv, dtype=np.float32),
        np.asarray(wo, dtype=np.float32), cos, sin)
    res = bass_utils.run_bass_kernel_spmd(
        nc, in_maps, core_ids=list(range(NC)), trace=_trace, tmpdir=_tmpdir)
    acc = np.zeros((B * S, D), dtype=np.float32)
    for c in range(NC):
        acc += res.results[c]["outp"].astype(np.float32)
    out = acc.reshape(B, S, D)
    if _trace:
        _CACHED["last_results"] = res
    return out


# revision 31
# speedup vs baseline: 1.3067x; 1.0474x over previous
"""Dense causal transformer attention block on 8 Trainium2 NeuronCores.

Problem: out = CausalAttention(RoPE(x@wq, x@wk), x@wv) @ wo
  x [2, 4096, 2048], 16 heads x 128 dim, fp32 I/O.

Sharding: tensor-parallel over heads. Core c owns heads {2c, 2c+1}:
  - computes qT/kT ([head_dim, seq] layout, w-stationary matmuls, RoPE
    on-chip) and V ([seq, head_dim] layout, x-stationary matmuls — no
    PE transpose needed) for its heads from the host-pre-transposed xT,
  - runs causal attention in transposed form (scoresT = k @ qT so the
    softmax weights come out as the moving operand of the A@V matmul),
  - denominators via an all-ones [128,128] stationary matmul (comes out
    pre-broadcast across partitions), fast approximate reciprocal,
  - computes its partial output projection o_local @ wo[rows of its heads].
Host sums the 8 partial outputs (the wo row-parallel all-reduce).

Software pipelining: the projection matmul chains for query tile i+1 are
emitted in small units interleaved into the attention pair loop of tile i,
so the PE has ready work while the scalar engine's exp() of each score
pair is still in flight. Causal masks are folded in as et multiplies
split across GpSimd and Vector; pair order puts one mask-free pair first
so the masked pairs' exp+mask latency hides behind it.

Compute dtype bf16 (PE 1 cycle/row), accumulation fp32 in PSUM.
"""
import sys

for _p in ("/opt/trn_rl_repo",):
    if _p not in sys.path:
        sys.path.insert(0, _p)

import numpy as np
import ml_dtypes
from contextlib import ExitStack

import concourse.bass as bass
import concourse.tile as tile
from concourse import bacc, mybir
from concourse import bass_utils

B, S, D = 2, 4096, 2048
H, DH = 16, 128
HALF = DH // 2
NC = 8
HPC = H // NC          # heads per core = 2
DOUT = HPC * DH        # 256 local proj width
ROPE_BASE = 10000.0
SCALE = 1.0 / float(np.sqrt(DH))
SQ = 512               # query tile (free dim of scoresT)
SKB = 128              # key block (partitions of scoresT)
KM = D // 128          # 16 contraction blocks
NSQ = S // SQ          # 8 query tiles per batch
NB = B * NSQ           # 16 bodies
BF = mybir.dt.bfloat16
F32 = mybir.dt.float32
INTERLEAVE = False

_CACHED = {}


def _build():
    nc = bacc.Bacc("TRN2", target_bir_lowering=False, debug=False, num_devices=NC)

    xT = nc.dram_tensor("xT", [D, B * S], BF, kind="ExternalInput").ap()
    wq = nc.dram_tensor("wq", [D, DOUT], BF, kind="ExternalInput").ap()
    wk = nc.dram_tensor("wk", [D, DOUT], BF, kind="ExternalInput").ap()
    wv = nc.dram_tensor("wv", [D, DOUT], BF, kind="ExternalInput").ap()
    wo = nc.dram_tensor("wo", [DOUT, D], BF, kind="ExternalInput").ap()
    cosf = nc.dram_tensor("cosf", [DH, S], BF, kind="ExternalInput").ap()
    sins = nc.dram_tensor("sins", [DH, S], BF, kind="ExternalInput").ap()
    masks = nc.dram_tensor("masks", [SKB, 4 * SQ], BF, kind="ExternalInput").ap()
    ones = nc.dram_tensor("ones", [128, 128], BF, kind="ExternalInput").ap()
    outp = nc.dram_tensor("outp", [B * S, D], BF, kind="ExternalOutput").ap()

    with tile.TileContext(nc) as tc, ExitStack() as ctx:
        const = ctx.enter_context(tc.tile_pool(name="const", bufs=1))
        xpool = ctx.enter_context(tc.tile_pool(name="xpool", bufs=4))
        qkv = ctx.enter_context(tc.tile_pool(name="qkv", bufs=1))
        rope = ctx.enter_context(tc.tile_pool(name="rope", bufs=2))
        attn = ctx.enter_context(tc.tile_pool(name="attn", bufs=4))
        opool = ctx.enter_context(tc.tile_pool(name="opool", bufs=2))

        wq_sb = const.tile([128, KM * DOUT], BF, name="wq_sb")
        wk_sb = const.tile([128, KM * DOUT], BF, name="wk_sb")
        wv_sb = const.tile([128, KM * DOUT], BF, name="wv_sb")
        ones_sb = const.tile([128, 128], BF, name="ones_sb")
        nc.sync.dma_start(ones_sb[:], ones[:])
        cos_sb = const.tile([DH, S], BF, name="cos_sb")
        sin_sb = const.tile([DH, S], BF, name="sin_sb")  # rows 0-63 = -sin
        mask_sb = const.tile([SKB, 4 * SQ], BF, name="mask_sb")
        wo_sb = const.tile([128, HPC * D], BF, name="wo_sb")    # [p, h*2048+n]

        qT = [qkv.tile([128, S], BF, tag=f"qT{j}", name=f"qT{j}") for j in range(HPC)]
        kT = [qkv.tile([128, S], BF, tag=f"kT{j}", name=f"kT{j}") for j in range(HPC)]
        vsb = [qkv.tile([128, S], BF, tag=f"v{j}", name=f"v{j}") for j in range(HPC)]
        oT = [qkv.tile([128, S], BF, tag=f"oT{j}", name=f"oT{j}") for j in range(HPC)]

        def flat(i):
            return (i // NSQ, i % NSQ)

        xbts = {}

        def emit_x_dma(i, eng=None, halves=False):
            if i >= NB or i in xbts:
                return None
            b_, t_ = flat(i)
            s0_ = t_ * SQ
            tiles = [xpool.tile([128, 8 * SQ], BF, tag=f"xb{hh}", bufs=2,
                                name=f"xbt{hh}") for hh in range(2)]
            xbts[i] = tiles
            e = eng or nc.sync
            if not halves:
                for hh in range(2):
                    e.dma_start(
                        tiles[hh][:].rearrange("p (a n) -> p a n", n=SQ),
                        xT[hh * 1024:(hh + 1) * 1024,
                           b_ * S + s0_: b_ * S + s0_ + SQ]
                        .rearrange("(a p) n -> p a n", p=128))
                return None

            # half-tile DMA closures the caller interleaves with the
            # weight DMAs (first proj matmuls start after ~0.5 MB)
            def part(hh, ha):
                e.dma_start(
                    tiles[hh][:, ha * 4 * SQ:(ha + 1) * 4 * SQ]
                    .rearrange("p (a n) -> p a n", n=SQ),
                    xT[hh * 1024 + ha * 512: hh * 1024 + (ha + 1) * 512,
                       b_ * S + s0_: b_ * S + s0_ + SQ]
                    .rearrange("(a p) n -> p a n", p=128))
            return part

        with tc.tile_pool(name="psm", bufs=1, space="PSUM") as psm:
            # PSUM budget (8 banks): pqk 1, pv 1, pscr 2x2, po 1, pd 1.
            # Out-proj pf tiles borrow the pscr slots.

            def proj_units(i):
                """Generator: proj PE work for body i in ~1us units.

                Unit order q(j)->v-sub->k(j)->v-sub keeps an independent
                PSUM bank between the chains that reuse the pqk bank, so
                the RoPE (DVE) read of the previous chain's accumulator
                never stalls the PE.
                """
                b_, t_ = flat(i)
                s0_ = t_ * SQ
                xbt = xbts[i]

                def qk_chain(j, w_sb, dstt):
                    pp = psm.tile([128, SQ], F32, tag="pqk", name="pp")
                    for u0 in range(0, KM, 4):
                        for km in range(u0, u0 + 4):
                            nc.tensor.matmul(
                                pp[:],
                                w_sb[:, km * DOUT + j * DH:
                                     km * DOUT + (j + 1) * DH],
                                xbt[km // 8][:, (km % 8) * SQ:
                                             (km % 8 + 1) * SQ],
                                start=km == 0, stop=km == KM - 1)
                        yield
                    rt = rope.tile([128, SQ], F32, tag="rot", name="rt")
                    nc.vector.tensor_mul(
                        rt[0:HALF, :], pp[HALF:128, :],
                        sin_sb[0:HALF, s0_:s0_ + SQ])
                    nc.vector.tensor_mul(
                        rt[HALF:128, :], pp[0:HALF, :],
                        sin_sb[HALF:128, s0_:s0_ + SQ])
                    m1 = rope.tile([128, SQ], F32, tag="m1", name="m1")
                    nc.vector.tensor_mul(m1[:], pp[:], cos_sb[:, s0_:s0_ + SQ])
                    nc.vector.tensor_add(dstt[:, s0_:s0_ + SQ], m1[:], rt[:])
                    yield

                def v_sub(sb):
                    # x-stationary: out = x_blk.T @ wv -> [seq 128, dh 256]
                    pvv = psm.tile([128, DOUT], F32, tag="pv", name="pvv")
                    for km in range(KM):
                        nc.tensor.matmul(
                            pvv[:],
                            xbt[km // 8][:, (km % 8) * SQ + sb * 128:
                                         (km % 8) * SQ + (sb + 1) * 128],
                            wv_sb[:, km * DOUT:(km + 1) * DOUT],
                            start=km == 0, stop=km == KM - 1)
                        if km % 8 == 7:
                            yield
                    blk = 4 * t_ + sb
                    for j in range(HPC):
                        dst = vsb[j][:, blk * 128:(blk + 1) * 128]
                        if j == 0:
                            nc.vector.tensor_copy(dst, pvv[:, 0:128])
                        else:
                            nc.scalar.copy(dst, pvv[:, 128:256])
                    yield

                yield from qk_chain(0, wq_sb, qT[0])
                yield from v_sub(0)
                yield from qk_chain(0, wk_sb, kT[0])
                yield from v_sub(1)
                yield from qk_chain(1, wq_sb, qT[1])
                yield from v_sub(2)
                yield from qk_chain(1, wk_sb, kT[1])
                yield from v_sub(3)

            def pull(gen, n):
                if gen is None:
                    return None
                for _ in range(n):
                    try:
                        next(gen)
                    except StopIteration:
                        return None
                return gen

            projected = set()
            for i_flat in range(NB):
                b, t = flat(i_flat)
                s0 = t * SQ
                if i_flat == 0:
                    # startup: interleave x(t0) halves with wq halves so the
                    # first proj chain starts after ~0.5 MB of DMA; the rest
                    # follows in need order.
                    xpart = emit_x_dma(0, halves=True)

                    def wq_half(wh):
                        nc.sync.dma_start(
                            wq_sb[:, wh * 8 * DOUT:(wh + 1) * 8 * DOUT]
                            .rearrange("p (a n) -> p a n", n=DOUT),
                            wq[wh * 1024:(wh + 1) * 1024, :]
                            .rearrange("(a p) n -> p a n", p=128))
                    xpart(0, 0)
                    wq_half(0)
                    xpart(0, 1)
                    wq_half(1)
                    xpart(1, 0)
                    xpart(1, 1)
                    nc.sync.dma_start(cos_sb[:], cosf[:])
                    nc.sync.dma_start(sin_sb[:], sins[:])
                    nc.sync.dma_start(
                        wv_sb[:].rearrange("p (a n) -> p a n", n=DOUT),
                        wv.rearrange("(a p) n -> p a n", p=128))
                    nc.sync.dma_start(
                        wk_sb[:].rearrange("p (a n) -> p a n", n=DOUT),
                        wk.rearrange("(a p) n -> p a n", p=128))
                    nc.sync.dma_start(mask_sb[:], masks[:])
                    emit_x_dma(1)
                    nc.sync.dma_start(
                        wo_sb[:].rearrange("p (a n) -> p a n", n=D),
                        wo.rearrange("(a p) n -> p a n", p=128))
                else:
                    emit_x_dma(i_flat + 1, eng=nc.scalar)

                if i_flat not in projected:
                    # proj not pipelined into the previous body (body 0,
                    # body 1, and each batch-boundary body whose s0=0
                    # writes would clobber regions the previous batch's
                    # last attention still reads) — run it eagerly.
                    g = proj_units(i_flat)
                    while pull(g, 1) is not None:
                        pass
                    xbts.pop(i_flat)
                    projected.add(i_flat)

                # proj of the NEXT body: emitted in units interleaved into
                # this body's attention pair loop (fills exp-latency
                # bubbles) — not across the batch boundary, and not out of
                # body 0 (the next body's x DMA is still behind the
                # startup constants).
                nxt = None
                nxt_i = i_flat + 1
                if (INTERLEAVE and i_flat != 0 and nxt_i < NB
                        and nxt_i % NSQ != 0):
                    nxt = proj_units(nxt_i)
                    projected.add(nxt_i)

                # --- causal attention for this query tile ---------------
                for j in range(HPC):
                    nblk = 4 * t + 4
                    npair = nblk // 2
                    nquad = npair // 2
                    po = psm.tile([128, SQ], F32, tag="po", name="po")
                    pd = psm.tile([128, SQ], F32, tag="pd", name="pd")
                    # one mask-free pair first (needs only the exp), then
                    # the diagonal (masked) pairs, then the rest
                    if t == 0:
                        order = [0, 1]
                    else:
                        order = [0, 2 * t, 2 * t + 1] + list(range(1, 2 * t))
                    prev_et = None
                    qs2_list = []
                    for idx, p in enumerate(order):
                        pscr = psm.tile([128, 2 * SQ], F32, tag="pscr",
                                        bufs=2, name="pscr")
                        for h in range(2):
                            u = 2 * p + h
                            nc.tensor.matmul(
                                pscr[:, h * SQ:(h + 1) * SQ],
                                kT[j][:, u * SKB:(u + 1) * SKB],
                                qT[j][:, s0:s0 + SQ], start=True, stop=True,
                                skip_group_check=True)
                        et = attn.tile([128, 2 * SQ], BF, tag="et", bufs=4,
                                       name="et")
                        nc.scalar.activation(
                            et[:], pscr[:], mybir.ActivationFunctionType.Exp,
                            scale=SCALE)
                        if 2 * p >= 4 * t:  # pair on the diagonal band
                            r = 2 * p - 4 * t   # 0 or 2
                            nc.gpsimd.tensor_mul(
                                et[:, 0:SQ], et[:, 0:SQ],
                                mask_sb[:, r * SQ:(r + 1) * SQ])
                            nc.vector.tensor_mul(
                                et[:, SQ:2 * SQ], et[:, SQ:2 * SQ],
                                mask_sb[:, (r + 1) * SQ:(r + 2) * SQ])
                        # PE filler while exp/mask are in flight
                        nxt = pull(nxt, 2 if idx == 0 else 1)
                        for h in range(2):
                            nc.tensor.matmul(
                                po[:], vsb[j][:, (2 * p + h) * 128:
                                              (2 * p + h + 1) * 128],
                                et[:, h * SQ:(h + 1) * SQ],
                                start=idx == 0 and h == 0,
                                stop=idx == npair - 1 and h == 1)
                        if idx % 2 == 1:
                            qs = attn.tile([128, 2 * SQ], BF, tag="qs",
                                           bufs=2, name="qs")
                            nc.vector.tensor_add(qs[:], prev_et[:], et[:])
                            qs2 = attn.tile([128, SQ], BF, tag="qs2",
                                            bufs=8, name="qs2")
                            nc.vector.tensor_add(
                                qs2[:], qs[:, 0:SQ], qs[:, SQ:2 * SQ])
                            qs2_list.append(qs2)
                        prev_et = et
                    # denominator matmuls batched at the end of the pair
                    # loop: inline they sit in the PE FIFO waiting on the
                    # DVE qs chain every quad; here only the last one can
                    # ever wait
                    for qi, qs2 in enumerate(qs2_list):
                        nc.tensor.matmul(
                            pd[:], ones_sb[:], qs2[:],
                            start=qi == 0, stop=qi == nquad - 1)
                    rec = attn.tile([128, SQ], F32, tag="rec", bufs=2,
                                    name="rec")
                    nc.vector.reciprocal_approx_fast(rec[:], pd[:])
                    nc.vector.tensor_mul(oT[j][:, s0:s0 + SQ], po[:], rec[:])

                # drain leftover proj units before the out-proj burst
                if nxt is not None:
                    while pull(nxt, 1) is not None:
                        pass
                    xbts.pop(nxt_i)

                # --- out-proj for the 4 seq blocks completed at t --------
                # pf tiles borrow the pscr tag's slots; the 4 n-blocks land
                # in one [128, 2048] SBUF tile -> one DMA per m-block.
                last_body = i_flat == NB - 1
                for m in range(4 * t, 4 * t + 4):
                    ob = opool.tile([128, D], BF, tag="ob", bufs=2,
                                    name="ob")
                    for n in range(D // 512):
                        pf = psm.tile([128, 512], F32, tag="pscr", bufs=2,
                                      name="pf")
                        for jj in range(HPC):
                            nc.tensor.matmul(
                                pf[:], oT[jj][:, m * 128:(m + 1) * 128],
                                wo_sb[:, jj * D + n * 512:
                                      jj * D + (n + 1) * 512],
                                start=jj == 0, stop=jj == HPC - 1)
                        if (m + n) % 2 == 0:
                            nc.vector.tensor_copy(
                                ob[:, n * 512:(n + 1) * 512], pf[:])
                        else:
                            nc.scalar.copy(
                                ob[:, n * 512:(n + 1) * 512], pf[:])
                        if last_body:
                            # tail: per-block DMAs overlap the remaining
                            # copies instead of waiting for all four
                            nc.sync.dma_start(
                                outp[b * S + m * 128: b * S + (m + 1) * 128,
                                     n * 512:(n + 1) * 512],
                                ob[:, n * 512:(n + 1) * 512])
                    if not last_body:
                        nc.sync.dma_start(
                            outp[b * S + m * 128: b * S + (m + 1) * 128, :],
                            ob[:])

    nc.compile()
    return nc


def _host_inputs(x, wq, wk, wv, wo, cos, sin):
    bf16 = ml_dtypes.bfloat16
    xT = np.ascontiguousarray(x.reshape(B * S, D).T).astype(bf16)

    cos = np.asarray(cos, dtype=np.float32)        # [S, 64]
    sin = np.asarray(sin, dtype=np.float32)
    cosf = np.ascontiguousarray(
        np.concatenate([cos, cos], axis=1).T).astype(bf16)   # [128, S]
    sins = np.concatenate([-sin, sin], axis=1).T   # rows 0-63 negated
    sins = np.ascontiguousarray(sins).astype(bf16)

    i = np.arange(SKB)[:, None]
    jj = np.arange(SQ)[None, :]
    masks = np.concatenate(
        [(i + r * SKB <= jj) for r in range(4)], axis=1).astype(bf16)
    ones = np.ones((128, 128), dtype=bf16)

    in_maps = []
    for c in range(NC):
        lo = c * DOUT
        in_maps.append({
            "xT": xT,
            "wq": np.ascontiguousarray(wq[:, lo:lo + DOUT]).astype(bf16),
            "wk": np.ascontiguousarray(wk[:, lo:lo + DOUT]).astype(bf16),
            "wv": np.ascontiguousarray(wv[:, lo:lo + DOUT]).astype(bf16),
            "wo": np.ascontiguousarray(wo[lo:lo + DOUT, :]).astype(bf16),
            "cosf": cosf,
            "sins": sins,
            "masks": masks,
            "ones": ones,
        })
    return in_maps


def kernel(x, wq, wk, wv, wo, cos, sin, _trace=False, _tmpdir=None):
    if "nc" not in _CACHED:
        _CACHED["nc"] = _build()
    nc = _CACHED["nc"]
    in_maps = _host_inputs(
        np.asarray(x, dtype=np.float32), np.asarray(wq, dtype=np.float32),
        np.asarray(wk, dtype=np.float32), np.asarray(wv, dtype=np.float32),
        np.asarray(wo, dtype=np.float32), cos, sin)
    res = bass_utils.run_bass_kernel_spmd(
        nc, in_maps, core_ids=list(range(NC)), trace=_trace, tmpdir=_tmpdir)
    acc = np.zeros((B * S, D), dtype=np.float32)
    for c in range(NC):
        acc += res.results[c]["outp"].astype(np.float32)
    out = acc.reshape(B, S, D)
    if _trace:
        _CACHED["last_results"] = res
    return out


# revision 36
# speedup vs baseline: 1.3118x; 1.0039x over previous
"""Dense causal transformer attention block on 8 Trainium2 NeuronCores.

Problem: out = CausalAttention(RoPE(x@wq, x@wk), x@wv) @ wo
  x [2, 4096, 2048], 16 heads x 128 dim, fp32 I/O.

Sharding: tensor-parallel over heads. Core c owns heads {2c, 2c+1}:
  - computes qT/kT ([head_dim, seq] layout, w-stationary matmuls, RoPE
    on-chip) and V ([seq, head_dim] layout, x-stationary matmuls — no
    PE transpose needed) for its heads from the host-pre-transposed xT,
  - runs causal attention in transposed form (scoresT = k @ qT so the
    softmax weights come out as the moving operand of the A@V matmul),
  - denominators via an all-ones [128,128] stationary matmul (comes out
    pre-broadcast across partitions), fast approximate reciprocal,
  - computes its partial output projection o_local @ wo[rows of its heads].
Host sums the 8 partial outputs (the wo row-parallel all-reduce).

Software pipelining: the projection matmul chains for query tile i+1 are
emitted in small units interleaved into the attention pair loop of tile i,
so the PE has ready work while the scalar engine's exp() of each score
pair is still in flight. Causal masks are folded in as et multiplies
split across GpSimd and Vector; pair order puts one mask-free pair first
so the masked pairs' exp+mask latency hides behind it.

Compute dtype bf16 (PE 1 cycle/row), accumulation fp32 in PSUM.
"""
import sys

for _p in ("/opt/trn_rl_repo",):
    if _p not in sys.path:
        sys.path.insert(0, _p)

import numpy as np
import ml_dtypes
from contextlib import ExitStack

import concourse.bass as bass
import concourse.tile as tile
from concourse import bacc, mybir
from concourse import bass_utils

B, S, D = 2, 4096, 2048
H, DH = 16, 128
HALF = DH // 2
NC = 8
HPC = H // NC          # heads per core = 2
DOUT = HPC * DH        # 256 local proj width
ROPE_BASE = 10000.0
SCALE = 1.0 / float(np.sqrt(DH))
SQ = 512               # query tile (free dim of scoresT)
SKB = 128              # key block (partitions of scoresT)
KM = D // 128          # 16 contraction blocks
NSQ = S // SQ          # 8 query tiles per batch
NB = B * NSQ           # 16 bodies
BF = mybir.dt.bfloat16
F32 = mybir.dt.float32
INTERLEAVE = True

_CACHED = {}


def _build():
    nc = bacc.Bacc("TRN2", target_bir_lowering=False, debug=False, num_devices=NC)

    xT = nc.dram_tensor("xT", [D, B * S], BF, kind="ExternalInput").ap()
    wq = nc.dram_tensor("wq", [D, DOUT], BF, kind="ExternalInput").ap()
    wk = nc.dram_tensor("wk", [D, DOUT], BF, kind="ExternalInput").ap()
    wv = nc.dram_tensor("wv", [D, DOUT], BF, kind="ExternalInput").ap()
    wo = nc.dram_tensor("wo", [DOUT, D], BF, kind="ExternalInput").ap()
    cosf = nc.dram_tensor("cosf", [DH, S], BF, kind="ExternalInput").ap()
    sins = nc.dram_tensor("sins", [DH, S], BF, kind="ExternalInput").ap()
    masks = nc.dram_tensor("masks", [SKB, 4 * SQ], BF, kind="ExternalInput").ap()
    ones = nc.dram_tensor("ones", [128, 128], BF, kind="ExternalInput").ap()
    outp = nc.dram_tensor("outp", [B * S, D], BF, kind="ExternalOutput").ap()

    with tile.TileContext(nc) as tc, ExitStack() as ctx:
        const = ctx.enter_context(tc.tile_pool(name="const", bufs=1))
        xpool = ctx.enter_context(tc.tile_pool(name="xpool", bufs=4))
        qkv = ctx.enter_context(tc.tile_pool(name="qkv", bufs=1))
        rope = ctx.enter_context(tc.tile_pool(name="rope", bufs=2))
        attn = ctx.enter_context(tc.tile_pool(name="attn", bufs=4))
        opool = ctx.enter_context(tc.tile_pool(name="opool", bufs=2))

        wq_sb = const.tile([128, KM * DOUT], BF, name="wq_sb")
        wk_sb = const.tile([128, KM * DOUT], BF, name="wk_sb")
        wv_sb = const.tile([128, KM * DOUT], BF, name="wv_sb")
        ones_sb = const.tile([128, 128], BF, name="ones_sb")
        nc.sync.dma_start(ones_sb[:], ones[:])
        cos_sb = const.tile([DH, S], BF, name="cos_sb")
        sin_sb = const.tile([DH, S], BF, name="sin_sb")  # rows 0-63 = -sin
        mask_sb = const.tile([SKB, 4 * SQ], BF, name="mask_sb")
        wo_sb = const.tile([128, HPC * D], BF, name="wo_sb")    # [p, h*2048+n]

        qT = [qkv.tile([128, S], BF, tag=f"qT{j}", name=f"qT{j}") for j in range(HPC)]
        kT = [qkv.tile([128, S], BF, tag=f"kT{j}", name=f"kT{j}") for j in range(HPC)]
        vsb = [qkv.tile([128, S], BF, tag=f"v{j}", name=f"v{j}") for j in range(HPC)]
        oT = [qkv.tile([128, S], BF, tag=f"oT{j}", name=f"oT{j}") for j in range(HPC)]

        def flat(i):
            return (i // NSQ, i % NSQ)

        xbts = {}

        def emit_x_dma(i, eng=None, halves=False):
            if i >= NB or i in xbts:
                return None
            b_, t_ = flat(i)
            s0_ = t_ * SQ
            tiles = [xpool.tile([128, 8 * SQ], BF, tag=f"xb{hh}", bufs=2,
                                name=f"xbt{hh}") for hh in range(2)]
            xbts[i] = tiles
            e = eng or nc.sync
            if not halves:
                for hh in range(2):
                    e.dma_start(
                        tiles[hh][:].rearrange("p (a n) -> p a n", n=SQ),
                        xT[hh * 1024:(hh + 1) * 1024,
                           b_ * S + s0_: b_ * S + s0_ + SQ]
                        .rearrange("(a p) n -> p a n", p=128))
                return None

            # half-tile DMA closures the caller interleaves with the
            # weight DMAs (first proj matmuls start after ~0.5 MB)
            def part(hh, ha):
                e.dma_start(
                    tiles[hh][:, ha * 4 * SQ:(ha + 1) * 4 * SQ]
                    .rearrange("p (a n) -> p a n", n=SQ),
                    xT[hh * 1024 + ha * 512: hh * 1024 + (ha + 1) * 512,
                       b_ * S + s0_: b_ * S + s0_ + SQ]
                    .rearrange("(a p) n -> p a n", p=128))
            return part

        with tc.tile_pool(name="psm", bufs=1, space="PSUM") as psm:
            # PSUM budget (8 banks): pqk 1, pv 1, pscr 2x2, po 1, pd 1.
            # Out-proj pf tiles borrow the pscr slots.

            def qk_units(i):
                """Generator: q/k proj chains for body i. Consecutive
                chains alternate between the pqk and pv PSUM banks so the
                RoPE (DVE) read of the previous chain's accumulator never
                stalls the next chain's first matmul (WAR on the bank)."""
                b_, t_ = flat(i)
                s0_ = t_ * SQ
                xbt = xbts[i]

                def qk_chain(j, w_sb, dstt, tag):
                    pp = psm.tile([128, SQ], F32, tag=tag, name="pp")
                    for u0 in range(0, KM, 4):
                        for km in range(u0, u0 + 4):
                            nc.tensor.matmul(
                                pp[:],
                                w_sb[:, km * DOUT + j * DH:
                                     km * DOUT + (j + 1) * DH],
                                xbt[km // 8][:, (km % 8) * SQ:
                                             (km % 8 + 1) * SQ],
                                start=km == 0, stop=km == KM - 1)
                        yield
                    rt = rope.tile([128, SQ], F32, tag="rot", name="rt")
                    nc.vector.tensor_mul(
                        rt[0:HALF, :], pp[HALF:128, :],
                        sin_sb[0:HALF, s0_:s0_ + SQ])
                    nc.vector.tensor_mul(
                        rt[HALF:128, :], pp[0:HALF, :],
                        sin_sb[HALF:128, s0_:s0_ + SQ])
                    m1 = rope.tile([128, SQ], F32, tag="m1", name="m1")
                    nc.vector.tensor_mul(m1[:], pp[:], cos_sb[:, s0_:s0_ + SQ])
                    nc.vector.tensor_add(dstt[:, s0_:s0_ + SQ], m1[:], rt[:])
                    yield

                yield from qk_chain(0, wq_sb, qT[0], "pqk")
                yield from qk_chain(0, wk_sb, kT[0], "pv")
                yield from qk_chain(1, wq_sb, qT[1], "pqk")
                yield from qk_chain(1, wk_sb, kT[1], "pv")

            def v_units(i):
                """Generator: V projection for body i (x-stationary, no
                PE transpose, and — unlike q/k — no DVE dependency before
                the matmuls), in ~0.9us units. Safe PE filler for the
                attention pair loop of body i-1."""
                b_, t_ = flat(i)
                xbt = xbts[i]
                for sb in range(4):
                    # out = x_blk.T @ wv -> [seq 128, dh 256]
                    pvv = psm.tile([128, DOUT], F32, tag="pv", name="pvv")
                    for km in range(KM):
                        nc.tensor.matmul(
                            pvv[:],
                            xbt[km // 8][:, (km % 8) * SQ + sb * 128:
                                         (km % 8) * SQ + (sb + 1) * 128],
                            wv_sb[:, km * DOUT:(km + 1) * DOUT],
                            start=km == 0, stop=km == KM - 1)
                        if km % 8 == 7:
                            yield
                    blk = 4 * t_ + sb
                    for j in range(HPC):
                        dst = vsb[j][:, blk * 128:(blk + 1) * 128]
                        if j == 0:
                            nc.vector.tensor_copy(dst, pvv[:, 0:128])
                        else:
                            nc.scalar.copy(dst, pvv[:, 128:256])
                    yield

            def pull(gen, n):
                if gen is None:
                    return None
                for _ in range(n):
                    try:
                        next(gen)
                    except StopIteration:
                        return None
                return gen

            v_done = set()
            for i_flat in range(NB):
                b, t = flat(i_flat)
                s0 = t * SQ
                if i_flat == 0:
                    # startup: interleave x(t0) halves with wq halves so the
                    # first proj chain starts after ~0.5 MB of DMA; the rest
                    # follows in need order.
                    xpart = emit_x_dma(0, halves=True)

                    def wq_half(wh):
                        nc.sync.dma_start(
                            wq_sb[:, wh * 8 * DOUT:(wh + 1) * 8 * DOUT]
                            .rearrange("p (a n) -> p a n", n=DOUT),
                            wq[wh * 1024:(wh + 1) * 1024, :]
                            .rearrange("(a p) n -> p a n", p=128))
                    xpart(0, 0)
                    wq_half(0)
                    xpart(0, 1)
                    wq_half(1)
                    xpart(1, 0)
                    xpart(1, 1)
                    nc.sync.dma_start(cos_sb[:], cosf[:])
                    nc.sync.dma_start(sin_sb[:], sins[:])
                    nc.sync.dma_start(
                        wv_sb[:].rearrange("p (a n) -> p a n", n=DOUT),
                        wv.rearrange("(a p) n -> p a n", p=128))
                    nc.sync.dma_start(
                        wk_sb[:].rearrange("p (a n) -> p a n", n=DOUT),
                        wk.rearrange("(a p) n -> p a n", p=128))
                    nc.sync.dma_start(mask_sb[:], masks[:])
                    emit_x_dma(1)
                    nc.sync.dma_start(
                        wo_sb[:].rearrange("p (a n) -> p a n", n=D),
                        wo.rearrange("(a p) n -> p a n", p=128))
                else:
                    emit_x_dma(i_flat + 1, eng=nc.scalar)

                # q/k proj of THIS body always runs here
                g = qk_units(i_flat)
                while pull(g, 1) is not None:
                    pass
                # V proj too, unless it was pipelined into the previous
                # body's attention
                if i_flat not in v_done:
                    g = v_units(i_flat)
                    while pull(g, 1) is not None:
                        pass
                    v_done.add(i_flat)
                xbts.pop(i_flat)

                # V proj of the NEXT body: interleaved into this body's
                # attention pair loop as PE filler while exp() is in
                # flight. V has no DVE dependency before its matmuls, so
                # it never blocks the PE FIFO. Not across the batch
                # boundary (its vsb writes for s0=0 would clobber blocks
                # this attention still reads), and not out of body 0 (x
                # DMA still behind startup constants).
                nxt = None
                nxt_i = i_flat + 1
                if (INTERLEAVE and i_flat != 0 and nxt_i < NB
                        and nxt_i % NSQ != 0):
                    nxt = v_units(nxt_i)
                    v_done.add(nxt_i)

                # --- causal attention for this query tile ---------------
                for j in range(HPC):
                    nblk = 4 * t + 4
                    npair = nblk // 2
                    nquad = npair // 2
                    po = psm.tile([128, SQ], F32, tag="po", name="po")
                    pd = psm.tile([128, SQ], F32, tag="pd", name="pd")
                    # one mask-free pair first (needs only the exp), then
                    # the diagonal (masked) pairs, then the rest
                    if t == 0:
                        order = [0, 1]
                    else:
                        order = [0, 2 * t, 2 * t + 1] + list(range(1, 2 * t))
                    prev_et = None
                    qs2_list = []
                    for idx, p in enumerate(order):
                        pscr = psm.tile([128, 2 * SQ], F32, tag="pscr",
                                        bufs=2, name="pscr")
                        for h in range(2):
                            u = 2 * p + h
                            nc.tensor.matmul(
                                pscr[:, h * SQ:(h + 1) * SQ],
                                kT[j][:, u * SKB:(u + 1) * SKB],
                                qT[j][:, s0:s0 + SQ], start=True, stop=True,
                                skip_group_check=True)
                        et = attn.tile([128, 2 * SQ], BF, tag="et", bufs=4,
                                       name="et")
                        nc.scalar.activation(
                            et[:], pscr[:], mybir.ActivationFunctionType.Exp,
                            scale=SCALE)
                        if 2 * p >= 4 * t:  # pair on the diagonal band
                            r = 2 * p - 4 * t   # 0 or 2
                            nc.gpsimd.tensor_mul(
                                et[:, 0:SQ], et[:, 0:SQ],
                                mask_sb[:, r * SQ:(r + 1) * SQ])
                            nc.vector.tensor_mul(
                                et[:, SQ:2 * SQ], et[:, SQ:2 * SQ],
                                mask_sb[:, (r + 1) * SQ:(r + 2) * SQ])
                        # PE filler while exp/mask are in flight
                        nxt = pull(nxt, 2 if idx == 0 else 1)
                        for h in range(2):
                            nc.tensor.matmul(
                                po[:], vsb[j][:, (2 * p + h) * 128:
                                              (2 * p + h + 1) * 128],
                                et[:, h * SQ:(h + 1) * SQ],
                                start=idx == 0 and h == 0,
                                stop=idx == npair - 1 and h == 1)
                        if idx % 2 == 1:
                            qs = attn.tile([128, 2 * SQ], BF, tag="qs",
                                           bufs=2, name="qs")
                            nc.vector.tensor_add(qs[:], prev_et[:], et[:])
                            qs2 = attn.tile([128, SQ], BF, tag="qs2",
                                            bufs=8, name="qs2")
                            nc.vector.tensor_add(
                                qs2[:], qs[:, 0:SQ], qs[:, SQ:2 * SQ])
                            qs2_list.append(qs2)
                        prev_et = et
                    # denominator matmuls batched at the end of the pair
                    # loop: inline they sit in the PE FIFO waiting on the
                    # DVE qs chain every quad; here only the last one can
                    # ever wait
                    for qi, qs2 in enumerate(qs2_list):
                        nc.tensor.matmul(
                            pd[:], ones_sb[:], qs2[:],
                            start=qi == 0, stop=qi == nquad - 1)
                    rec = attn.tile([128, SQ], F32, tag="rec", bufs=2,
                                    name="rec")
                    nc.vector.reciprocal_approx_fast(rec[:], pd[:])
                    nc.vector.tensor_mul(oT[j][:, s0:s0 + SQ], po[:], rec[:])

                # drain leftover V units before the out-proj burst
                while pull(nxt, 1) is not None:
                    pass

                # --- out-proj for the 4 seq blocks completed at t --------
                # pf tiles borrow the pscr tag's slots; the 4 n-blocks land
                # in one [128, 2048] SBUF tile -> one DMA per m-block.
                last_body = i_flat == NB - 1
                for m in range(4 * t, 4 * t + 4):
                    ob = opool.tile([128, D], BF, tag="ob", bufs=2,
                                    name="ob")
                    for n in range(D // 512):
                        pf = psm.tile([128, 512], F32, tag="pscr", bufs=2,
                                      name="pf")
                        for jj in range(HPC):
                            nc.tensor.matmul(
                                pf[:], oT[jj][:, m * 128:(m + 1) * 128],
                                wo_sb[:, jj * D + n * 512:
                                      jj * D + (n + 1) * 512],
                                start=jj == 0, stop=jj == HPC - 1)
                        if (m + n) % 2 == 0:
                            nc.vector.tensor_copy(
                                ob[:, n * 512:(n + 1) * 512], pf[:])
                        else:
                            nc.scalar.copy(
                                ob[:, n * 512:(n + 1) * 512], pf[:])
                        if last_body:
                            # tail: per-block DMAs overlap the remaining
                            # copies instead of waiting for all four
                            nc.sync.dma_start(
                                outp[b * S + m * 128: b * S + (m + 1) * 128,
                                     n * 512:(n + 1) * 512],
                                ob[:, n * 512:(n + 1) * 512])
                    if not last_body:
                        nc.sync.dma_start(
                            outp[b * S + m * 128: b * S + (m + 1) * 128, :],
                            ob[:])

    nc.compile()
    return nc


def _host_inputs(x, wq, wk, wv, wo, cos, sin):
    bf16 = ml_dtypes.bfloat16
    xT = np.ascontiguousarray(x.reshape(B * S, D).T).astype(bf16)

    cos = np.asarray(cos, dtype=np.float32)        # [S, 64]
    sin = np.asarray(sin, dtype=np.float32)
    cosf = np.ascontiguousarray(
        np.concatenate([cos, cos], axis=1).T).astype(bf16)   # [128, S]
    sins = np.concatenate([-sin, sin], axis=1).T   # rows 0-63 negated
    sins = np.ascontiguousarray(sins).astype(bf16)

    i = np.arange(SKB)[:, None]
    jj = np.arange(SQ)[None, :]
    masks = np.concatenate(
        [(i + r * SKB <= jj) for r in range(4)], axis=1).astype(bf16)
    ones = np.ones((128, 128), dtype=bf16)

    in_maps = []
    for c in range(NC):
        lo = c * DOUT
        in_maps.append({
            "xT": xT,
            "wq": np.ascontiguousarray(wq[:, lo:lo + DOUT]).astype(bf16),
            "wk": np.ascontiguousarray(wk[:, lo:lo + DOUT]).astype(bf16),
            "wv": np.ascontiguousarray(wv[:, lo:lo + DOUT]).astype(bf16),
            "wo": np.ascontiguousarray(wo[lo:lo + DOUT, :]).astype(bf16),
            "cosf": cosf,
            "sins": sins,
            "masks": masks,
            "ones": ones,
        })
    return in_maps


def kernel(x, wq, wk, wv, wo, cos, sin, _trace=False, _tmpdir=None):
    if "nc" not in _CACHED:
        _CACHED["nc"] = _build()
    nc = _CACHED["nc"]
    in_maps = _host_inputs(
        np.asarray(x, dtype=np.float32), np.asarray(wq, dtype=np.float32),
        np.asarray(wk, dtype=np.float32), np.asarray(wv, dtype=np.float32),
        np.asarray(wo, dtype=np.float32), cos, sin)
    res = bass_utils.run_bass_kernel_spmd(
        nc, in_maps, core_ids=list(range(NC)), trace=_trace, tmpdir=_tmpdir)
    acc = np.zeros((B * S, D), dtype=np.float32)
    for c in range(NC):
        acc += res.results[c]["outp"].astype(np.float32)
    out = acc.reshape(B, S, D)
    if _trace:
        _CACHED["last_results"] = res
    return out


# revision 42
# speedup vs baseline: 1.3844x; 1.0554x over previous
"""Dense causal transformer attention block on 8 Trainium2 NeuronCores.

Problem: out = CausalAttention(RoPE(x@wq, x@wk), x@wv) @ wo
  x [2, 4096, 2048], 16 heads x 128 dim, fp32 I/O.

Sharding: tensor-parallel over heads. Core c owns heads {2c, 2c+1}:
  - computes qT/kT ([head_dim, seq] layout, w-stationary matmuls, RoPE
    on-chip) and V ([seq, head_dim] layout, x-stationary matmuls — no
    PE transpose needed) for its heads from the host-pre-transposed xT,
  - runs causal attention in transposed form (scoresT = k @ qT so the
    softmax weights come out as the moving operand of the A@V matmul),
  - denominators via an all-ones [128,128] stationary matmul (comes out
    pre-broadcast across partitions), fast approximate reciprocal,
  - computes its partial output projection o_local @ wo[rows of its heads].
Host sums the 8 partial outputs (the wo row-parallel all-reduce).

Software pipelining: the projection matmul chains for query tile i+1 are
emitted in small units interleaved into the attention pair loop of tile i,
so the PE has ready work while the scalar engine's exp() of each score
pair is still in flight. Causal masks are folded in as et multiplies
split across GpSimd and Vector; pair order puts one mask-free pair first
so the masked pairs' exp+mask latency hides behind it.

Compute dtype bf16 (PE 1 cycle/row), accumulation fp32 in PSUM.
"""
import sys

for _p in ("/opt/trn_rl_repo",):
    if _p not in sys.path:
        sys.path.insert(0, _p)

import numpy as np
import ml_dtypes
from contextlib import ExitStack

import concourse.bass as bass
import concourse.tile as tile
from concourse import bacc, mybir
from concourse import bass_utils

B, S, D = 2, 4096, 2048
H, DH = 16, 128
HALF = DH // 2
NC = 8
HPC = H // NC          # heads per core = 2
DOUT = HPC * DH        # 256 local proj width
ROPE_BASE = 10000.0
SCALE = 1.0 / float(np.sqrt(DH))
SQ = 512               # query tile (free dim of scoresT)
SKB = 128              # key block (partitions of scoresT)
KM = D // 128          # 16 contraction blocks
NSQ = S // SQ          # 8 query tiles per batch
NB = B * NSQ           # 16 bodies
BF = mybir.dt.bfloat16
F32 = mybir.dt.float32
INTERLEAVE = True

_CACHED = {}


def _build():
    nc = bacc.Bacc("TRN2", target_bir_lowering=False, debug=False, num_devices=NC)

    # All inputs host-pre-arranged into their exact SBUF layouts so every
    # DMA is a flat contiguous [128, n] copy (8 KB runs per partition —
    # fewer descriptors, full HBM burst efficiency).
    xr = nc.dram_tensor("xr", [128, NB * 2 * 8 * SQ], BF,
                        kind="ExternalInput").ap()
    wq = nc.dram_tensor("wq", [128, KM * DOUT], BF, kind="ExternalInput").ap()
    wk = nc.dram_tensor("wk", [128, KM * DOUT], BF, kind="ExternalInput").ap()
    wv = nc.dram_tensor("wv", [128, KM * DOUT], BF, kind="ExternalInput").ap()
    wo = nc.dram_tensor("wo", [128, HPC * D], BF, kind="ExternalInput").ap()
    cosf = nc.dram_tensor("cosf", [DH, S], BF, kind="ExternalInput").ap()
    sins = nc.dram_tensor("sins", [DH, S], BF, kind="ExternalInput").ap()
    masks = nc.dram_tensor("masks", [SKB, SKB], BF, kind="ExternalInput").ap()
    ones = nc.dram_tensor("ones", [128, 128], BF, kind="ExternalInput").ap()
    outp = nc.dram_tensor("outp", [B * S, D], BF, kind="ExternalOutput").ap()

    with tile.TileContext(nc) as tc, ExitStack() as ctx:
        const = ctx.enter_context(tc.tile_pool(name="const", bufs=1))
        xpool = ctx.enter_context(tc.tile_pool(name="xpool", bufs=4))
        qkv = ctx.enter_context(tc.tile_pool(name="qkv", bufs=1))
        rope = ctx.enter_context(tc.tile_pool(name="rope", bufs=2))
        attn = ctx.enter_context(tc.tile_pool(name="attn", bufs=4))
        opool = ctx.enter_context(tc.tile_pool(name="opool", bufs=2))

        wq_sb = const.tile([128, KM * DOUT], BF, name="wq_sb")
        wk_sb = const.tile([128, KM * DOUT], BF, name="wk_sb")
        wv_sb = const.tile([128, KM * DOUT], BF, name="wv_sb")
        ones_sb = const.tile([128, 128], BF, name="ones_sb")
        nc.sync.dma_start(ones_sb[:], ones[:])
        cos_sb = const.tile([DH, S], BF, name="cos_sb")
        sin_sb = const.tile([DH, S], BF, name="sin_sb")  # rows 0-63 = -sin
        mask_sb = const.tile([SKB, SKB], BF, name="mask_sb")
        wo_sb = const.tile([128, HPC * D], BF, name="wo_sb")    # [p, h*2048+n]

        qT = [qkv.tile([128, S], BF, tag=f"qT{j}", name=f"qT{j}") for j in range(HPC)]
        kT = [qkv.tile([128, S], BF, tag=f"kT{j}", name=f"kT{j}") for j in range(HPC)]
        vsb = [qkv.tile([128, S], BF, tag=f"v{j}", name=f"v{j}") for j in range(HPC)]
        oT = [qkv.tile([128, S], BF, tag=f"oT{j}", name=f"oT{j}") for j in range(HPC)]

        def flat(i):
            return (i // NSQ, i % NSQ)

        xbts = {}

        def emit_x_dma(i, eng=None, halves=False):
            if i >= NB or i in xbts:
                return None
            b_, t_ = flat(i)
            s0_ = t_ * SQ
            tiles = [xpool.tile([128, 8 * SQ], BF, tag=f"xb{hh}", bufs=2,
                                name=f"xbt{hh}") for hh in range(2)]
            xbts[i] = tiles
            e = eng or nc.sync
            base = i * 2 * 8 * SQ
            if not halves:
                for hh in range(2):
                    e.dma_start(
                        tiles[hh][:],
                        xr[:, base + hh * 8 * SQ: base + (hh + 1) * 8 * SQ])
                return None

            # half-tile DMA closures the caller interleaves with the
            # weight DMAs (first proj matmuls start after ~0.5 MB)
            def part(hh, ha):
                off = base + hh * 8 * SQ + ha * 4 * SQ
                e.dma_start(
                    tiles[hh][:, ha * 4 * SQ:(ha + 1) * 4 * SQ],
                    xr[:, off: off + 4 * SQ])
            return part

        with tc.tile_pool(name="psm", bufs=1, space="PSUM") as psm:
            # PSUM budget (8 banks): pqk 1, pv 1, pscr 2x2, po 1, pd 1.
            # Out-proj pf tiles borrow the pscr slots.

            def qk_units(i):
                """Generator: q/k proj chains for body i. Consecutive
                chains alternate between the pqk and pv PSUM banks so the
                RoPE (DVE) read of the previous chain's accumulator never
                stalls the next chain's first matmul (WAR on the bank)."""
                b_, t_ = flat(i)
                s0_ = t_ * SQ
                xbt = xbts[i]

                def qk_chain(j, w_sb, dstt, tag):
                    pp = psm.tile([128, SQ], F32, tag=tag, name="pp")
                    for u0 in range(0, KM, 4):
                        for km in range(u0, u0 + 4):
                            nc.tensor.matmul(
                                pp[:],
                                w_sb[:, km * DOUT + j * DH:
                                     km * DOUT + (j + 1) * DH],
                                xbt[km // 8][:, (km % 8) * SQ:
                                             (km % 8 + 1) * SQ],
                                start=km == 0, stop=km == KM - 1)
                        yield
                    rt = rope.tile([128, SQ], F32, tag="rot", name="rt")
                    nc.vector.tensor_mul(
                        rt[0:HALF, :], pp[HALF:128, :],
                        sin_sb[0:HALF, s0_:s0_ + SQ])
                    nc.vector.tensor_mul(
                        rt[HALF:128, :], pp[0:HALF, :],
                        sin_sb[HALF:128, s0_:s0_ + SQ])
                    m1 = rope.tile([128, SQ], F32, tag="m1", name="m1")
                    nc.vector.tensor_mul(m1[:], pp[:], cos_sb[:, s0_:s0_ + SQ])
                    nc.vector.tensor_add(dstt[:, s0_:s0_ + SQ], m1[:], rt[:])
                    yield

                yield from qk_chain(0, wq_sb, qT[0], "pqk")
                yield from qk_chain(0, wk_sb, kT[0], "pv")
                yield from qk_chain(1, wq_sb, qT[1], "pqk")
                yield from qk_chain(1, wk_sb, kT[1], "pv")

            def v_units(i):
                """Generator: V projection for body i (x-stationary, no
                PE transpose, and — unlike q/k — no DVE dependency before
                the matmuls), in ~0.9us units. Safe PE filler for the
                attention pair loop of body i-1."""
                b_, t_ = flat(i)
                xbt = xbts[i]
                for sb in range(4):
                    # out = x_blk.T @ wv -> [seq 128, dh 256]
                    pvv = psm.tile([128, DOUT], F32, tag="pv", name="pvv")
                    for km in range(KM):
                        nc.tensor.matmul(
                            pvv[:],
                            xbt[km // 8][:, (km % 8) * SQ + sb * 128:
                                         (km % 8) * SQ + (sb + 1) * 128],
                            wv_sb[:, km * DOUT:(km + 1) * DOUT],
                            start=km == 0, stop=km == KM - 1)
                        if km % 8 == 7:
                            yield
                    blk = 4 * t_ + sb
                    for j in range(HPC):
                        dst = vsb[j][:, blk * 128:(blk + 1) * 128]
                        if j == 0:
                            nc.vector.tensor_copy(dst, pvv[:, 0:128])
                        else:
                            nc.scalar.copy(dst, pvv[:, 128:256])
                    yield

            def pull(gen, n):
                if gen is None:
                    return None
                for _ in range(n):
                    try:
                        next(gen)
                    except StopIteration:
                        return None
                return gen

            v_done = set()
            for i_flat in range(NB):
                b, t = flat(i_flat)
                s0 = t * SQ
                if i_flat == 0:
                    # startup: interleave x(t0) halves with wq halves so the
                    # first proj chain starts after ~0.5 MB of DMA; the rest
                    # follows in need order.
                    xpart = emit_x_dma(0, halves=True)

                    def wq_half(wh):
                        nc.sync.dma_start(
                            wq_sb[:, wh * 8 * DOUT:(wh + 1) * 8 * DOUT],
                            wq[:, wh * 8 * DOUT:(wh + 1) * 8 * DOUT])
                    xpart(0, 0)
                    wq_half(0)
                    xpart(0, 1)
                    wq_half(1)
                    xpart(1, 0)
                    xpart(1, 1)
                    # rope/tile-0 constants in need order; full cos/sin
                    # tails come after x(t1)
                    nc.sync.dma_start(cos_sb[:, 0:SQ], cosf[:, 0:SQ])
                    nc.sync.dma_start(sin_sb[:, 0:SQ], sins[:, 0:SQ])
                    nc.sync.dma_start(wk_sb[:], wk[:])
                    nc.sync.dma_start(mask_sb[:], masks[:])
                    nc.sync.dma_start(wv_sb[:], wv[:])
                    emit_x_dma(1)
                    nc.sync.dma_start(cos_sb[:, SQ:], cosf[:, SQ:])
                    nc.sync.dma_start(sin_sb[:, SQ:], sins[:, SQ:])
                    nc.sync.dma_start(wo_sb[:], wo[:])
                else:
                    emit_x_dma(i_flat + 1, eng=nc.scalar)

                # q/k proj of THIS body always runs here
                g = qk_units(i_flat)
                while pull(g, 1) is not None:
                    pass
                # V proj too, unless it was pipelined into the previous
                # body's attention
                if i_flat not in v_done:
                    g = v_units(i_flat)
                    while pull(g, 1) is not None:
                        pass
                    v_done.add(i_flat)
                xbts.pop(i_flat)

                # V proj of the NEXT body: interleaved into this body's
                # attention pair loop as PE filler while exp() is in
                # flight. V has no DVE dependency before its matmuls, so
                # it never blocks the PE FIFO. Not across the batch
                # boundary (its vsb writes for s0=0 would clobber blocks
                # this attention still reads), and not out of body 0 (x
                # DMA still behind startup constants).
                nxt = None
                nxt_i = i_flat + 1
                if (INTERLEAVE and i_flat != 0 and nxt_i < NB
                        and nxt_i % NSQ != 0):
                    nxt = v_units(nxt_i)
                    v_done.add(nxt_i)

                # --- causal attention for this query tile ---------------
                for j in range(HPC):
                    nblk = 4 * t + 4
                    npair = nblk // 2
                    nquad = npair // 2
                    po = psm.tile([128, SQ], F32, tag="po", name="po")
                    pd = psm.tile([128, SQ], F32, tag="pd", name="pd")
                    # one mask-free pair first (needs only the exp), then
                    # the diagonal (masked) pairs, then the rest
                    if t == 0:
                        order = [0, 1]
                    else:
                        order = [0, 2 * t, 2 * t + 1] + list(range(1, 2 * t))
                    prev_et = None
                    qs2_list = []
                    for idx, p in enumerate(order):
                        diag = 2 * p >= 4 * t
                        pscr = psm.tile([128, 2 * SQ], F32, tag="pscr",
                                        bufs=2, name="pscr")
                        for h in range(2):
                            u = 2 * p + h
                            qo = (u - 4 * t) * 128 if diag else 0
                            nc.tensor.matmul(
                                pscr[:, h * SQ + qo:(h + 1) * SQ],
                                kT[j][:, u * SKB:(u + 1) * SKB],
                                qT[j][:, s0 + qo:s0 + SQ],
                                start=True, stop=True,
                                skip_group_check=True)
                        et = attn.tile([128, 2 * SQ], BF, tag="et", bufs=4,
                                       name="et")
                        if not diag:
                            nc.scalar.activation(
                                et[:], pscr[:],
                                mybir.ActivationFunctionType.Exp, scale=SCALE)
                        else:
                            # exact causal: key block u is attended only by
                            # queries qo.. within the tile; zero the rest of
                            # et, exp the live range, triangular-mask the
                            # first 128 columns of it
                            for h in range(2):
                                u = 2 * p + h
                                qo = (u - 4 * t) * 128
                                if qo > 0:
                                    nc.vector.memset(
                                        et[:, h * SQ:h * SQ + qo], 0)
                                nc.scalar.activation(
                                    et[:, h * SQ + qo:(h + 1) * SQ],
                                    pscr[:, h * SQ + qo:(h + 1) * SQ],
                                    mybir.ActivationFunctionType.Exp,
                                    scale=SCALE)
                                eng = nc.gpsimd if h == 0 else nc.vector
                                eng.tensor_mul(
                                    et[:, h * SQ + qo:h * SQ + qo + 128],
                                    et[:, h * SQ + qo:h * SQ + qo + 128],
                                    mask_sb[:])
                        # PE filler while exp/mask are in flight
                        nxt = pull(nxt, 2 if idx == 0 else 1)
                        for h in range(2):
                            u = 2 * p + h
                            qo = (u - 4 * t) * 128 if diag else 0
                            nc.tensor.matmul(
                                po[:, qo:SQ], vsb[j][:, u * 128:
                                                     (u + 1) * 128],
                                et[:, h * SQ + qo:(h + 1) * SQ],
                                start=idx == 0 and h == 0,
                                stop=idx == npair - 1 and h == 1)
                        if idx % 2 == 1:
                            qs = attn.tile([128, 2 * SQ], BF, tag="qs",
                                           bufs=2, name="qs")
                            nc.vector.tensor_add(qs[:], prev_et[:], et[:])
                            qs2 = attn.tile([128, SQ], BF, tag="qs2",
                                            bufs=8, name="qs2")
                            nc.vector.tensor_add(
                                qs2[:], qs[:, 0:SQ], qs[:, SQ:2 * SQ])
                            qs2_list.append(qs2)
                        prev_et = et
                    # denominator matmuls batched at the end of the pair
                    # loop: inline they sit in the PE FIFO waiting on the
                    # DVE qs chain every quad; here only the last one can
                    # ever wait
                    for qi, qs2 in enumerate(qs2_list):
                        nc.tensor.matmul(
                            pd[:], ones_sb[:], qs2[:],
                            start=qi == 0, stop=qi == nquad - 1)
                    rec = attn.tile([128, SQ], F32, tag="rec", bufs=2,
                                    name="rec")
                    nc.vector.reciprocal_approx_fast(rec[:], pd[:])
                    nc.vector.tensor_mul(oT[j][:, s0:s0 + SQ], po[:], rec[:])

                # drain leftover V units before the out-proj burst
                while pull(nxt, 1) is not None:
                    pass

                # --- out-proj for the 4 seq blocks completed at t --------
                # pf tiles borrow the pscr tag's slots; the 4 n-blocks land
                # in one [128, 2048] SBUF tile -> one DMA per m-block.
                last_body = i_flat == NB - 1
                for m in range(4 * t, 4 * t + 4):
                    ob = opool.tile([128, D], BF, tag="ob", bufs=2,
                                    name="ob")
                    for n in range(D // 512):
                        pf = psm.tile([128, 512], F32, tag="pscr", bufs=2,
                                      name="pf")
                        for jj in range(HPC):
                            nc.tensor.matmul(
                                pf[:], oT[jj][:, m * 128:(m + 1) * 128],
                                wo_sb[:, jj * D + n * 512:
                                      jj * D + (n + 1) * 512],
                                start=jj == 0, stop=jj == HPC - 1)
                        if (m + n) % 2 == 0:
                            nc.vector.tensor_copy(
                                ob[:, n * 512:(n + 1) * 512], pf[:])
                        else:
                            nc.scalar.copy(
                                ob[:, n * 512:(n + 1) * 512], pf[:])
                        if last_body:
                            # tail: per-block DMAs overlap the remaining
                            # copies instead of waiting for all four
                            nc.sync.dma_start(
                                outp[b * S + m * 128: b * S + (m + 1) * 128,
                                     n * 512:(n + 1) * 512],
                                ob[:, n * 512:(n + 1) * 512])
                    if not last_body:
                        nc.sync.dma_start(
                            outp[b * S + m * 128: b * S + (m + 1) * 128, :],
                            ob[:])

    nc.compile()
    return nc


def _host_inputs(x, wq, wk, wv, wo, cos, sin):
    bf16 = ml_dtypes.bfloat16

    # x in the exact SBUF tile layout: xr[p, ((b,t), hh, a, n)] =
    # x[b, t*SQ + n, hh*1024 + a*128 + p]
    x5 = np.asarray(x, dtype=np.float32).reshape(B, NSQ, SQ, 2, 8, 128)
    xr = np.ascontiguousarray(
        x5.transpose(5, 0, 1, 3, 4, 2).reshape(128, -1)).astype(bf16)

    def wpack(w):   # [D, DOUT] -> [128, km*DOUT], w_sb[p, km*DOUT+o]
        return np.ascontiguousarray(
            w.reshape(KM, 128, DOUT).transpose(1, 0, 2)
            .reshape(128, KM * DOUT)).astype(bf16)

    def wopack(w):  # [DOUT, D] -> [128, jj*D+n]
        return np.ascontiguousarray(
            w.reshape(HPC, 128, D).transpose(1, 0, 2)
            .reshape(128, HPC * D)).astype(bf16)

    cos = np.asarray(cos, dtype=np.float32)        # [S, 64]
    sin = np.asarray(sin, dtype=np.float32)
    cosf = np.ascontiguousarray(
        np.concatenate([cos, cos], axis=1).T).astype(bf16)   # [128, S]
    sins = np.concatenate([-sin, sin], axis=1).T   # rows 0-63 negated
    sins = np.ascontiguousarray(sins).astype(bf16)

    i = np.arange(SKB)[:, None]
    jj = np.arange(SKB)[None, :]
    masks = (i <= jj).astype(bf16)
    ones = np.ones((128, 128), dtype=bf16)

    in_maps = []
    for c in range(NC):
        lo = c * DOUT
        in_maps.append({
            "xr": xr,
            "wq": wpack(wq[:, lo:lo + DOUT]),
            "wk": wpack(wk[:, lo:lo + DOUT]),
            "wv": wpack(wv[:, lo:lo + DOUT]),
            "wo": wopack(wo[lo:lo + DOUT, :]),
            "cosf": cosf,
            "sins": sins,
            "masks": masks,
            "ones": ones,
        })
    return in_maps


def kernel(x, wq, wk, wv, wo, cos, sin, _trace=False, _tmpdir=None):
    if "nc" not in _CACHED:
        _CACHED["nc"] = _build()
    nc = _CACHED["nc"]
    in_maps = _host_inputs(
        np.asarray(x, dtype=np.float32), np.asarray(wq, dtype=np.float32),
        np.asarray(wk, dtype=np.float32), np.asarray(wv, dtype=np.float32),
        np.asarray(wo, dtype=np.float32), cos, sin)
    res = bass_utils.run_bass_kernel_spmd(
        nc, in_maps, core_ids=list(range(NC)), trace=_trace, tmpdir=_tmpdir)
    acc = np.zeros((B * S, D), dtype=np.float32)
    for c in range(NC):
        acc += res.results[c]["outp"].astype(np.float32)
    out = acc.reshape(B, S, D)
    if _trace:
        _CACHED["last_results"] = res
    return out


# revision 49
# speedup vs baseline: 1.4024x; 1.0131x over previous
"""Dense causal transformer attention block on 8 Trainium2 NeuronCores.

Problem: out = CausalAttention(RoPE(x@wq, x@wk), x@wv) @ wo
  x [2, 4096, 2048], 16 heads x 128 dim, fp32 I/O.

Sharding: tensor-parallel over heads. Core c owns heads {2c, 2c+1}:
  - computes qT/kT ([head_dim, seq] layout, w-stationary matmuls, RoPE
    on-chip) and V ([seq, head_dim] layout, x-stationary matmuls — no
    PE transpose needed) for its heads from the host-pre-transposed xT,
  - runs causal attention in transposed form (scoresT = k @ qT so the
    softmax weights come out as the moving operand of the A@V matmul),
  - denominators via an all-ones [128,128] stationary matmul (comes out
    pre-broadcast across partitions), fast approximate reciprocal,
  - computes its partial output projection o_local @ wo[rows of its heads].
Host sums the 8 partial outputs (the wo row-parallel all-reduce).

Software pipelining: the projection matmul chains for query tile i+1 are
emitted in small units interleaved into the attention pair loop of tile i,
so the PE has ready work while the scalar engine's exp() of each score
pair is still in flight. Causal masks are folded in as et multiplies
split across GpSimd and Vector; pair order puts one mask-free pair first
so the masked pairs' exp+mask latency hides behind it.

Compute dtype bf16 (PE 1 cycle/row), accumulation fp32 in PSUM.
"""
import sys

for _p in ("/opt/trn_rl_repo",):
    if _p not in sys.path:
        sys.path.insert(0, _p)

import numpy as np
import ml_dtypes
from contextlib import ExitStack

import concourse.bass as bass
import concourse.tile as tile
from concourse import bacc, mybir
from concourse import bass_utils

B, S, D = 2, 4096, 2048
H, DH = 16, 128
HALF = DH // 2
NC = 8
HPC = H // NC          # heads per core = 2
DOUT = HPC * DH        # 256 local proj width
ROPE_BASE = 10000.0
SCALE = 1.0 / float(np.sqrt(DH))
SQ = 512               # query tile (free dim of scoresT)
SKB = 128              # key block (partitions of scoresT)
KM = D // 128          # 16 contraction blocks
NSQ = S // SQ          # 8 query tiles per batch
NB = B * NSQ           # 16 bodies
BF = mybir.dt.bfloat16
F32 = mybir.dt.float32
INTERLEAVE = True

_CACHED = {}


def _build():
    nc = bacc.Bacc("TRN2", target_bir_lowering=False, debug=False, num_devices=NC)

    # All inputs host-pre-arranged into their exact SBUF layouts so every
    # DMA is a flat contiguous [128, n] copy (8 KB runs per partition —
    # fewer descriptors, full HBM burst efficiency).
    xr = nc.dram_tensor("xr", [128, NB * 2 * 8 * SQ], BF,
                        kind="ExternalInput").ap()
    wq = nc.dram_tensor("wq", [128, KM * DOUT], BF, kind="ExternalInput").ap()
    wk = nc.dram_tensor("wk", [128, KM * DOUT], BF, kind="ExternalInput").ap()
    wv = nc.dram_tensor("wv", [128, KM * DOUT], BF, kind="ExternalInput").ap()
    wo = nc.dram_tensor("wo", [128, HPC * D], BF, kind="ExternalInput").ap()
    cosf = nc.dram_tensor("cosf", [DH, S], BF, kind="ExternalInput").ap()
    sins = nc.dram_tensor("sins", [DH, S], BF, kind="ExternalInput").ap()
    masks = nc.dram_tensor("masks", [SKB, SKB], BF, kind="ExternalInput").ap()
    ones = nc.dram_tensor("ones", [128, 128], BF, kind="ExternalInput").ap()
    outp = nc.dram_tensor("outp", [B * S, D], BF, kind="ExternalOutput").ap()

    with tile.TileContext(nc) as tc, ExitStack() as ctx:
        const = ctx.enter_context(tc.tile_pool(name="const", bufs=1))
        xpool = ctx.enter_context(tc.tile_pool(name="xpool", bufs=4))
        qkv = ctx.enter_context(tc.tile_pool(name="qkv", bufs=1))
        rope = ctx.enter_context(tc.tile_pool(name="rope", bufs=2))
        attn = ctx.enter_context(tc.tile_pool(name="attn", bufs=4))
        opool = ctx.enter_context(tc.tile_pool(name="opool", bufs=2))

        wq_sb = const.tile([128, KM * DOUT], BF, name="wq_sb")
        wk_sb = const.tile([128, KM * DOUT], BF, name="wk_sb")
        wv_sb = const.tile([128, KM * DOUT], BF, name="wv_sb")
        ones_sb = const.tile([128, 128], BF, name="ones_sb")
        nc.sync.dma_start(ones_sb[:], ones[:])
        cos_sb = const.tile([DH, S], BF, name="cos_sb")
        sin_sb = const.tile([DH, S], BF, name="sin_sb")  # rows 0-63 = -sin
        mask_sb = const.tile([SKB, SKB], BF, name="mask_sb")
        wo_sb = const.tile([128, HPC * D], BF, name="wo_sb")    # [p, h*2048+n]

        qT = [qkv.tile([128, S], BF, tag=f"qT{j}", name=f"qT{j}") for j in range(HPC)]
        kT = [qkv.tile([128, S], BF, tag=f"kT{j}", name=f"kT{j}") for j in range(HPC)]
        vsb = [qkv.tile([128, S], BF, tag=f"v{j}", name=f"v{j}") for j in range(HPC)]
        oT = [qkv.tile([128, S], BF, tag=f"oT{j}", name=f"oT{j}") for j in range(HPC)]

        def flat(i):
            return (i // NSQ, i % NSQ)

        xbts = {}

        def emit_x_dma(i, eng=None, halves=False):
            if i >= NB or i in xbts:
                return None
            b_, t_ = flat(i)
            s0_ = t_ * SQ
            tiles = [xpool.tile([128, 8 * SQ], BF, tag=f"xb{hh}", bufs=2,
                                name=f"xbt{hh}") for hh in range(2)]
            xbts[i] = tiles
            e = eng or nc.sync
            base = i * 2 * 8 * SQ
            if not halves:
                for hh in range(2):
                    e.dma_start(
                        tiles[hh][:],
                        xr[:, base + hh * 8 * SQ: base + (hh + 1) * 8 * SQ])
                return None

            # quarter-tile DMA closures the caller interleaves with the
            # weight DMAs (first proj matmuls start after ~0.75 MB)
            def part(hh, qa):
                off = base + hh * 8 * SQ + qa * 2 * SQ
                e.dma_start(
                    tiles[hh][:, qa * 2 * SQ:(qa + 1) * 2 * SQ],
                    xr[:, off: off + 2 * SQ])
            return part

        with tc.tile_pool(name="psm", bufs=1, space="PSUM") as psm:
            # PSUM budget (8 banks): pqk 1, pv 1, pscr 2x2, po 1, pd 1.
            # Out-proj pf tiles borrow the pscr slots.

            def qk_units(i):
                """Generator: q/k proj chains for body i. Consecutive
                chains alternate between the pqk and pv PSUM banks so the
                RoPE (DVE) read of the previous chain's accumulator never
                stalls the next chain's first matmul (WAR on the bank)."""
                b_, t_ = flat(i)
                s0_ = t_ * SQ
                xbt = xbts[i]

                def qk_chain(j, w_sb, dstt, tag):
                    pp = psm.tile([128, SQ], F32, tag=tag, name="pp")
                    for u0 in range(0, KM, 4):
                        for km in range(u0, u0 + 4):
                            nc.tensor.matmul(
                                pp[:],
                                w_sb[:, km * DOUT + j * DH:
                                     km * DOUT + (j + 1) * DH],
                                xbt[km // 8][:, (km % 8) * SQ:
                                             (km % 8 + 1) * SQ],
                                start=km == 0, stop=km == KM - 1)
                        yield
                    rt = rope.tile([128, SQ], F32, tag="rot", name="rt")
                    nc.vector.tensor_mul(
                        rt[0:HALF, :], pp[HALF:128, :],
                        sin_sb[0:HALF, s0_:s0_ + SQ])
                    nc.vector.tensor_mul(
                        rt[HALF:128, :], pp[0:HALF, :],
                        sin_sb[HALF:128, s0_:s0_ + SQ])
                    m1 = rope.tile([128, SQ], F32, tag="m1", name="m1")
                    nc.vector.tensor_mul(m1[:], pp[:], cos_sb[:, s0_:s0_ + SQ])
                    nc.vector.tensor_add(dstt[:, s0_:s0_ + SQ], m1[:], rt[:])
                    yield

                yield from qk_chain(0, wq_sb, qT[0], "pqk")
                yield from qk_chain(0, wk_sb, kT[0], "pv")
                yield from qk_chain(1, wq_sb, qT[1], "pqk")
                yield from qk_chain(1, wk_sb, kT[1], "pv")

            def v_units(i):
                """Generator: V projection for body i (x-stationary, no
                PE transpose, and — unlike q/k — no DVE dependency before
                the matmuls), in ~0.9us units. Safe PE filler for the
                attention pair loop of body i-1."""
                b_, t_ = flat(i)
                xbt = xbts[i]
                for sb in range(4):
                    # out = x_blk.T @ wv -> [seq 128, dh 256]
                    pvv = psm.tile([128, DOUT], F32, tag="pv", name="pvv")
                    for km in range(KM):
                        nc.tensor.matmul(
                            pvv[:],
                            xbt[km // 8][:, (km % 8) * SQ + sb * 128:
                                         (km % 8) * SQ + (sb + 1) * 128],
                            wv_sb[:, km * DOUT:(km + 1) * DOUT],
                            start=km == 0, stop=km == KM - 1)
                        if km % 8 == 7:
                            yield
                    blk = 4 * t_ + sb
                    for j in range(HPC):
                        dst = vsb[j][:, blk * 128:(blk + 1) * 128]
                        if j == 0:
                            nc.vector.tensor_copy(dst, pvv[:, 0:128])
                        else:
                            nc.scalar.copy(dst, pvv[:, 128:256])
                    yield

            def pull(gen, n):
                if gen is None:
                    return None
                for _ in range(n):
                    try:
                        next(gen)
                    except StopIteration:
                        return None
                return gen

            v_done = set()
            for i_flat in range(NB):
                b, t = flat(i_flat)
                s0 = t * SQ
                if i_flat == 0:
                    # startup: interleave x(t0) halves with wq halves so the
                    # first proj chain starts after ~0.5 MB of DMA; the rest
                    # follows in need order.
                    xpart = emit_x_dma(0, halves=True)

                    def wq_q(wh):
                        nc.sync.dma_start(
                            wq_sb[:, wh * 4 * DOUT:(wh + 1) * 4 * DOUT],
                            wq[:, wh * 4 * DOUT:(wh + 1) * 4 * DOUT])
                    xpart(0, 0)
                    wq_q(0)
                    xpart(0, 1)
                    xpart(0, 2)
                    wq_q(1)
                    xpart(0, 3)
                    xpart(1, 0)
                    wq_q(2)
                    xpart(1, 1)
                    xpart(1, 2)
                    wq_q(3)
                    xpart(1, 3)
                    # rope/tile-0 constants in need order; full cos/sin
                    # tails come after x(t1)
                    nc.sync.dma_start(cos_sb[:, 0:SQ], cosf[:, 0:SQ])
                    nc.sync.dma_start(sin_sb[:, 0:SQ], sins[:, 0:SQ])
                    nc.sync.dma_start(wk_sb[:], wk[:])
                    nc.sync.dma_start(mask_sb[:], masks[:])
                    nc.sync.dma_start(wv_sb[:], wv[:])
                    emit_x_dma(1)
                    nc.sync.dma_start(cos_sb[:, SQ:], cosf[:, SQ:])
                    nc.sync.dma_start(sin_sb[:, SQ:], sins[:, SQ:])
                    nc.sync.dma_start(wo_sb[:], wo[:])
                else:
                    emit_x_dma(i_flat + 1, eng=nc.scalar)

                # q/k proj of THIS body always runs here
                g = qk_units(i_flat)
                while pull(g, 1) is not None:
                    pass
                # V proj too, unless it was pipelined into the previous
                # body's attention
                if i_flat not in v_done:
                    g = v_units(i_flat)
                    while pull(g, 1) is not None:
                        pass
                    v_done.add(i_flat)
                xbts.pop(i_flat)

                # V proj of the NEXT body: interleaved into this body's
                # attention pair loop as PE filler while exp() is in
                # flight. V has no DVE dependency before its matmuls, so
                # it never blocks the PE FIFO. Not across the batch
                # boundary (its vsb writes for s0=0 would clobber blocks
                # this attention still reads), and not out of body 0 (x
                # DMA still behind startup constants).
                nxt = None
                bnxt = None
                nxt_i = i_flat + 1
                if INTERLEAVE and i_flat != 0 and nxt_i < NB:
                    if nxt_i % NSQ != 0:
                        nxt = v_units(nxt_i)
                        v_done.add(nxt_i)
                    else:
                        # batch boundary: v(nxt_i) writes vsb blocks 0-3,
                        # which THIS attention's pairs p=0,1 still read.
                        # Safe to pull only in the last head's loop after
                        # idx>=4 (both heads' pairs 0,1 are then emitted,
                        # so the WAR deps resolve without stalling).
                        bnxt = v_units(nxt_i)
                        v_done.add(nxt_i)

                # --- causal attention for this query tile ---------------
                for j in range(HPC):
                    nblk = 4 * t + 4
                    npair = nblk // 2
                    nquad = npair // 2
                    po = psm.tile([128, SQ], F32, tag="po", name="po")
                    pd = psm.tile([128, SQ], F32, tag="pd", name="pd")
                    # mask-free pairs first (need only the exp), then the
                    # diagonal (masked) pairs whose longer exp+mask chains
                    # hide behind the rest
                    if t == 0:
                        order = [0, 1]
                    elif t == 1:
                        order = [0, 1, 2, 3]
                    else:
                        order = ([0, 1, 2 * t, 2 * t + 1]
                                 + list(range(2, 2 * t)))
                    prev_et = None
                    qs2_list = []
                    for idx, p in enumerate(order):
                        diag = 2 * p >= 4 * t
                        pscr = psm.tile([128, 2 * SQ], F32, tag="pscr",
                                        bufs=2, name="pscr")
                        for h in range(2):
                            u = 2 * p + h
                            qo = (u - 4 * t) * 128 if diag else 0
                            nc.tensor.matmul(
                                pscr[:, h * SQ + qo:(h + 1) * SQ],
                                kT[j][:, u * SKB:(u + 1) * SKB],
                                qT[j][:, s0 + qo:s0 + SQ],
                                start=True, stop=True,
                                skip_group_check=True)
                        et = attn.tile([128, 2 * SQ], BF, tag="et", bufs=4,
                                       name="et")
                        if not diag:
                            nc.scalar.activation(
                                et[:], pscr[:],
                                mybir.ActivationFunctionType.Exp, scale=SCALE)
                        else:
                            # exact causal: key block u is attended only by
                            # queries qo.. within the tile; zero the rest of
                            # et, exp the live range, triangular-mask the
                            # first 128 columns of it
                            for h in range(2):
                                u = 2 * p + h
                                qo = (u - 4 * t) * 128
                                if qo > 0:
                                    nc.vector.memset(
                                        et[:, h * SQ:h * SQ + qo], 0)
                                nc.scalar.activation(
                                    et[:, h * SQ + qo:(h + 1) * SQ],
                                    pscr[:, h * SQ + qo:(h + 1) * SQ],
                                    mybir.ActivationFunctionType.Exp,
                                    scale=SCALE)
                                eng = nc.gpsimd if h == 0 else nc.vector
                                eng.tensor_mul(
                                    et[:, h * SQ + qo:h * SQ + qo + 128],
                                    et[:, h * SQ + qo:h * SQ + qo + 128],
                                    mask_sb[:])
                        # PE filler while exp/mask are in flight
                        if nxt is not None:
                            nxt = pull(nxt, 2 if idx == 0 else 1)
                        elif bnxt is not None and j == HPC - 1 and idx >= 4:
                            bnxt = pull(bnxt, 1)
                        for h in range(2):
                            u = 2 * p + h
                            qo = (u - 4 * t) * 128 if diag else 0
                            nc.tensor.matmul(
                                po[:, qo:SQ], vsb[j][:, u * 128:
                                                     (u + 1) * 128],
                                et[:, h * SQ + qo:(h + 1) * SQ],
                                start=idx == 0 and h == 0,
                                stop=idx == npair - 1 and h == 1)
                        if idx % 2 == 1:
                            qs = attn.tile([128, 2 * SQ], BF, tag="qs",
                                           bufs=2, name="qs")
                            nc.vector.tensor_add(qs[:], prev_et[:], et[:])
                            qs2 = attn.tile([128, SQ], BF, tag="qs2",
                                            bufs=8, name="qs2")
                            nc.vector.tensor_add(
                                qs2[:], qs[:, 0:SQ], qs[:, SQ:2 * SQ])
                            qs2_list.append(qs2)
                        prev_et = et
                    # denominator matmuls batched at the end of the pair
                    # loop: inline they sit in the PE FIFO waiting on the
                    # DVE qs chain every quad; here only the last one can
                    # ever wait
                    for qi, qs2 in enumerate(qs2_list):
                        nc.tensor.matmul(
                            pd[:], ones_sb[:], qs2[:],
                            start=qi == 0, stop=qi == nquad - 1)
                    rec = attn.tile([128, SQ], F32, tag="rec", bufs=2,
                                    name="rec")
                    nc.vector.reciprocal_approx_fast(rec[:], pd[:])
                    nc.vector.tensor_mul(oT[j][:, s0:s0 + SQ], po[:], rec[:])

                # drain leftover V units before the out-proj burst
                while pull(nxt, 1) is not None:
                    pass
                while pull(bnxt, 1) is not None:
                    pass

                # --- out-proj for the 4 seq blocks completed at t --------
                # pf tiles borrow the pscr tag's slots; the 4 n-blocks land
                # in one [128, 2048] SBUF tile -> one DMA per m-block.
                last_body = i_flat == NB - 1
                for m in range(4 * t, 4 * t + 4):
                    ob = opool.tile([128, D], BF, tag="ob", bufs=2,
                                    name="ob")
                    for n2 in range(2):
                        # [128,1024] pf = 2 PSUM banks; with bufs=2 the
                        # evacuation (690ns/bank) double-buffers behind
                        # the fills (432ns/bank) instead of throttling
                        pf = psm.tile([128, 1024], F32, tag="pscr", bufs=2,
                                      name="pf")
                        for nn in range(2):
                            n = n2 * 2 + nn
                            for jj in range(HPC):
                                nc.tensor.matmul(
                                    pf[:, nn * 512:(nn + 1) * 512],
                                    oT[jj][:, m * 128:(m + 1) * 128],
                                    wo_sb[:, jj * D + n * 512:
                                          jj * D + (n + 1) * 512],
                                    start=jj == 0, stop=jj == HPC - 1)
                        if (m + n2) % 2 == 0:
                            nc.vector.tensor_copy(
                                ob[:, n2 * 1024:(n2 + 1) * 1024], pf[:])
                        else:
                            nc.scalar.copy(
                                ob[:, n2 * 1024:(n2 + 1) * 1024], pf[:])
                        if last_body:
                            # tail: per-block DMAs overlap the remaining
                            # copies instead of waiting for all
                            nc.sync.dma_start(
                                outp[b * S + m * 128: b * S + (m + 1) * 128,
                                     n2 * 1024:(n2 + 1) * 1024],
                                ob[:, n2 * 1024:(n2 + 1) * 1024])
                    if not last_body:
                        nc.sync.dma_start(
                            outp[b * S + m * 128: b * S + (m + 1) * 128, :],
                            ob[:])

    nc.compile()
    return nc


def _host_inputs(x, wq, wk, wv, wo, cos, sin):
    bf16 = ml_dtypes.bfloat16

    # x in the exact SBUF tile layout: xr[p, ((b,t), hh, a, n)] =
    # x[b, t*SQ + n, hh*1024 + a*128 + p]
    x5 = np.asarray(x, dtype=np.float32).reshape(B, NSQ, SQ, 2, 8, 128)
    xr = np.ascontiguousarray(
        x5.transpose(5, 0, 1, 3, 4, 2).reshape(128, -1)).astype(bf16)

    def wpack(w):   # [D, DOUT] -> [128, km*DOUT], w_sb[p, km*DOUT+o]
        return np.ascontiguousarray(
            w.reshape(KM, 128, DOUT).transpose(1, 0, 2)
            .reshape(128, KM * DOUT)).astype(bf16)

    def wopack(w):  # [DOUT, D] -> [128, jj*D+n]
        return np.ascontiguousarray(
            w.reshape(HPC, 128, D).transpose(1, 0, 2)
            .reshape(128, HPC * D)).astype(bf16)

    cos = np.asarray(cos, dtype=np.float32)        # [S, 64]
    sin = np.asarray(sin, dtype=np.float32)
    cosf = np.ascontiguousarray(
        np.concatenate([cos, cos], axis=1).T).astype(bf16)   # [128, S]
    sins = np.concatenate([-sin, sin], axis=1).T   # rows 0-63 negated
    sins = np.ascontiguousarray(sins).astype(bf16)

    i = np.arange(SKB)[:, None]
    jj = np.arange(SKB)[None, :]
    masks = (i <= jj).astype(bf16)
    ones = np.ones((128, 128), dtype=bf16)

    in_maps = []
    for c in range(NC):
        lo = c * DOUT
        in_maps.append({
            "xr": xr,
            "wq": wpack(wq[:, lo:lo + DOUT]),
            "wk": wpack(wk[:, lo:lo + DOUT]),
            "wv": wpack(wv[:, lo:lo + DOUT]),
            "wo": wopack(wo[lo:lo + DOUT, :]),
            "cosf": cosf,
            "sins": sins,
            "masks": masks,
            "ones": ones,
        })
    return in_maps


def kernel(x, wq, wk, wv, wo, cos, sin, _trace=False, _tmpdir=None):
    if "nc" not in _CACHED:
        _CACHED["nc"] = _build()
    nc = _CACHED["nc"]
    in_maps = _host_inputs(
        np.asarray(x, dtype=np.float32), np.asarray(wq, dtype=np.float32),
        np.asarray(wk, dtype=np.float32), np.asarray(wv, dtype=np.float32),
        np.asarray(wo, dtype=np.float32), cos, sin)
    res = bass_utils.run_bass_kernel_spmd(
        nc, in_maps, core_ids=list(range(NC)), trace=_trace, tmpdir=_tmpdir)
    acc = np.zeros((B * S, D), dtype=np.float32)
    for c in range(NC):
        acc += res.results[c]["outp"].astype(np.float32)
    out = acc.reshape(B, S, D)
    if _trace:
        _CACHED["last_results"] = res
    return out
